# revision 2
# baseline (speedup 1.0000x reference)
"""GravityAE GNN message-passing kernel for 8 TRN2 NeuronCores (Bass/Tile), v3.

Math (GCN autoencoder, eval):
  scale_k = gamma_k/sqrt(var_k+eps); shift_k = beta_k + (b_k-mean_k)*scale_k
  Wkp = W_k*scale_k;  dinv[n] = 1/sqrt(in_deg incl self loop)
  xs = dinv*x  (fp16 DRAM table);  agg1[d] = sum_{e:dst=d} xs[src]
  h  = leaky(dinv_d*agg1 @ W1p + shift1);  hs = dinv*h  (fp16 table)
  z  = leaky(dinv_d*(sum hs[src]) @ W2p + shift2)
  out[e] = sigmoid(z[dst,64] - ||z[src,:64]-z[dst,:64]||)

Distribution: dst-sharded aggregation (49 windows of 128 nodes per core;
edges+self-loops sorted by dst, then split per window into two compacted
streams by src half since dma_gather indices are int16). Gathers use the
Pool-engine dma_gather ucode (one call per 4-window batch per half,
single_packet=False) which amortizes the ~1us SWDGE fixed cost over
thousands of descriptors. Segment-sum = S^T @ msg in PSUM with S built
by one fp16 is_equal per window; dense W matmul on the PE-transposed
window; leaky-relu as max(dinv*v, 0.1*dinv*v) with the scales applied by
the Act engine. AllGather (x8) rebuilds the hs and z tables between
stages; z is stored padded to 128 cols so decode rows are 256B. Decode
is edge-sharded, edges classed by (src-half, dst-half) so each 2-group
batch needs two dma_gathers; the host inverse-permutes the output.
"""
import numpy as np

P = 128
EPS = 1e-5
HALF = 32768          # int16 index limit for dma_gather
GK = 3                # windows per layer gather batch
DB = 2                # decode groups per gather batch


# --------------------------------------------------------------------------
# host-side preprocessing
# --------------------------------------------------------------------------
def _idx16(idx_flat, cols):
    """int16 index tile [128, cols]: slot i -> [i%16, i//16], replicated x8."""
    t = np.zeros((16, cols), np.int16)
    n = len(idx_flat)
    t[np.arange(n) % 16, np.arange(n) // 16] = idx_flat.astype(np.int16)
    return np.tile(t, (8, 1))


def _build_host_tables(x, edge_index, n_cores):
    N = x.shape[0]
    E = edge_index.shape[1]
    NW = ((N + P - 1) // P + n_cores - 1) // n_cores * n_cores
    NP_ = NW * P
    src = edge_index[0].astype(np.int64)
    dst = edge_index[1].astype(np.int64)
    s_all = np.concatenate([src, np.arange(N)])
    d_all = np.concatenate([dst, np.arange(N)])
    deg = np.bincount(d_all, minlength=NP_).astype(np.float64)
    dinv = np.zeros(NP_, np.float32)
    nz = deg > 0
    dinv[nz] = (1.0 / np.sqrt(deg[nz])).astype(np.float32)

    # sort by (dst window, src-half): gives per-window contiguous A then B runs
    win = d_all // P
    half = (s_all >= HALF).astype(np.int64)
    order = np.lexsort((half, d_all))
    s_sorted = s_all[order]
    d_sorted = d_all[order]
    h_sorted = half[order]

    cntAB = np.bincount(win[order] * 2 + h_sorted, minlength=NW * 2).reshape(NW, 2)
    NWc = NW // n_cores
    cpc = cntAB.reshape(n_cores, NWc, 2)
    CAw = np.maximum(1, np.ceil(cpc[:, :, 0].max(0) / P).astype(np.int64))  # [NWc]
    CBw = np.maximum(1, np.ceil(cpc[:, :, 1].max(0) / P).astype(np.int64))
    CTw = CAw + CBw
    CTmax = int(CTw.max())
    doff = np.zeros(NWc + 1, np.int64)
    np.cumsum(CTw, out=doff[1:])                      # dstf col offsets

    # per-window slot streams (per core, window-slot i): A then B
    starts = np.zeros(NW + 1, np.int64)
    np.cumsum(cntAB.sum(1), out=starts[1:])

    nbat = (NWc + GK - 1) // GK
    batches = [min(GK, NWc - b * GK) for b in range(nbat)]
    batA = [int(CAw[b * GK:b * GK + k].sum()) for b, k in enumerate(batches)]
    batB = [int(CBw[b * GK:b * GK + k].sum()) for b, k in enumerate(batches)]

    idxA_core, idxB_core, dstf_core, dv_core = [], [], [], []
    dv = dinv.reshape(NW, P).T                        # [P, NW]
    for c in range(n_cores):
        segsA, segsB = [], []
        dstf_c = np.full((int(doff[-1]), P), -1.0, np.float16)  # [cols, P]
        for b, k in enumerate(batches):
            ia = np.zeros(batA[b] * P, np.int64)
            ib = np.zeros(batB[b] * P, np.int64)
            oa = ob = 0
            for i in range(b * GK, b * GK + k):
                W = c * NWc + i
                sl = slice(starts[W], starts[W + 1])
                ss = s_sorted[sl]
                dd = d_sorted[sl] - W * P
                hh = h_sorted[sl]
                a, bb = ss[hh == 0], ss[hh == 1] - HALF
                ia[oa:oa + len(a)] = a
                ib[ob:ob + len(bb)] = bb
                da, db_ = dd[hh == 0], dd[hh == 1]
                col0 = doff[i]
                dstf_c.reshape(-1)[col0 * P:col0 * P + len(da)] = da.astype(np.float16)
                colB = (doff[i] + CAw[i]) * P
                dstf_c.reshape(-1)[colB:colB + len(db_)] = db_.astype(np.float16)
                oa += CAw[i] * P
                ob += CBw[i] * P
            segsA.append(_idx16(ia, batA[b] * 8))
            segsB.append(_idx16(ib, batB[b] * 8))
        idxA_core.append(np.concatenate(segsA, 1))
        idxB_core.append(np.concatenate(segsB, 1))
        dstf_core.append(np.ascontiguousarray(dstf_c.T))   # [P, sum(CTw)]
        dv_core.append(np.ascontiguousarray(dv[:, c * NWc:(c + 1) * NWc]))

    # ---- decode: 4 classes by (src-half, dst-half), groups of 2048 ----
    EC = (E + n_cores - 1) // n_cores
    cls_all = (src >= HALF) * 2 + (dst >= HALF)
    gcls = np.zeros((n_cores, 4), np.int64)
    per_core = []
    for c in range(n_cores):
        e0, e1 = c * EC, min((c + 1) * EC, E)
        ids = [np.nonzero(cls_all[e0:e1] == k)[0] for k in range(4)]
        gcls[c] = [(len(i) + 2047) // 2048 for i in ids]
        per_core.append((e0, ids))
    gmax = gcls.max(0)                                # uniform group counts
    gmax = np.maximum(gmax, 1)
    # pad each class to a DB multiple so every DB-batch is single-class
    gmax = (gmax + DB - 1) // DB * DB
    DGT = int(gmax.sum())
    g0 = np.zeros(5, np.int64)
    np.cumsum(gmax, out=g0[1:])

    dsrc_core, ddst_core, perm_core = [], [], []
    for c in range(n_cores):
        e0, ids = per_core[c]
        sflat = np.zeros(DGT * 2048, np.int64)
        dflat = np.zeros(DGT * 2048, np.int64)
        perm = []                                     # (slot, local_edge_id)
        for k in range(4):
            base = g0[k] * 2048
            ek = ids[k]
            sk = src[e0 + ek]
            dk = dst[e0 + ek]
            sflat[base:base + len(ek)] = sk - (HALF if k >= 2 else 0)
            dflat[base:base + len(ek)] = dk - (HALF if k % 2 else 0)
            perm.append((base, ek))
        dsrc_core.append(_idx16(sflat, DGT * 128))
        ddst_core.append(_idx16(dflat, DGT * 128))
        perm_core.append(perm)

    return dict(N=N, E=E, NW=NW, NP=NP_, CAw=CAw, CBw=CBw, CTmax=CTmax,
                doff=doff, batA=batA, batB=batB, NWc=NWc,
                EC=EC, DGT=DGT, batches=batches, cls_bases=g0,
                dinv=dinv, idxA_core=idxA_core, idxB_core=idxB_core,
                dstf_core=dstf_core, dv_core=dv_core,
                dsrc_core=dsrc_core, ddst_core=ddst_core,
                perm_core=perm_core)


# --------------------------------------------------------------------------
# bass program
# --------------------------------------------------------------------------
def _build_program(NP_, NWc, CAw, CBw, CTmax, doff, batA, batB, F1, F2, DGT,
                   batches, cls_bases, n_cores):
    import concourse.bass as bass
    import concourse.tile as tile
    from concourse import bacc, mybir

    dt = mybir.dt
    f32 = dt.float32
    f16 = dt.float16
    i16 = dt.int16
    Nc = NWc * P
    SA, SB, ST = sum(batA), sum(batB), int(doff[-1])
    mA_max, mB_max = max(batA), max(batB)
    Fz = 128                      # z stored padded to 128 cols (256B rows)
    NB = NP_ - HALF               # rows in half B

    nc = bacc.Bacc("TRN2", target_bir_lowering=False, debug=False,
                   num_devices=n_cores)
    xs_in = nc.declare_dram_parameter("xs", [NP_, F1], f16, isOutput=False)
    w1_in = nc.declare_dram_parameter("w1", [F1, F1], f16, isOutput=False)
    w2_in = nc.declare_dram_parameter("w2", [F1, F2], f16, isOutput=False)
    sh1_in = nc.declare_dram_parameter("shift1", [P, F1], f32, isOutput=False)
    sh2_in = nc.declare_dram_parameter("shift2", [P, F2], f32, isOutput=False)
    iota_in = nc.declare_dram_parameter("iota", [P, CTmax * P], f16, isOutput=False)
    idh_in = nc.declare_dram_parameter("identh", [P, P], f16, isOutput=False)
    dv_in = nc.declare_dram_parameter("dv", [P, NWc], f32, isOutput=False)
    dv01_in = nc.declare_dram_parameter("dv01", [P, NWc], f32, isOutput=False)
    ixa_in = nc.declare_dram_parameter("idxA", [P, SA * 8], i16, isOutput=False)
    ixb_in = nc.declare_dram_parameter("idxB", [P, SB * 8], i16, isOutput=False)
    dstf_in = nc.declare_dram_parameter("dstf", [P, ST], f16, isOutput=False)
    dsrc_in = nc.declare_dram_parameter("dsrc", [P, DGT * 128], i16, isOutput=False)
    ddst_in = nc.declare_dram_parameter("ddst", [P, DGT * 128], i16, isOutput=False)
    out_dram = nc.declare_dram_parameter("out", [P, DGT * 16], f32, isOutput=True)

    rg = [list(range(n_cores))]
    AF = mybir.ActivationFunctionType
    OP = mybir.AluOpType

    with tile.TileContext(nc) as tc:
        with (
            tc.tile_pool(name="const", bufs=1) as cpool,
            tc.tile_pool(name="msgp", bufs=2) as mpool,
            tc.tile_pool(name="sbuf", bufs=3) as pool,
            tc.tile_pool(name="dec", bufs=2) as dpool2,
            tc.tile_pool(name="psA", bufs=2, space="PSUM") as psA,
            tc.tile_pool(name="dram", bufs=1, space="DRAM") as dpool,
        ):
            # ---- constants ----
            w1_t = cpool.tile([F1, F1], f16)
            w2_t = cpool.tile([F1, F2], f16)
            sh1_t = cpool.tile([P, F1], f32)
            sh2_t = cpool.tile([P, F2], f32)
            iota_t = cpool.tile([P, CTmax * P], f16)
            idh_t = cpool.tile([P, P], f16)
            dv_t = cpool.tile([P, NWc], f32)
            dv01_t = cpool.tile([P, NWc], f32)
            ixa_t = cpool.tile([P, SA * 8], i16)
            ixb_t = cpool.tile([P, SB * 8], i16)
            dstf_t = cpool.tile([P, ST], f16)
            for t_, p_ in ((w1_t, w1_in), (w2_t, w2_in), (sh1_t, sh1_in),
                           (sh2_t, sh2_in), (iota_t, iota_in), (idh_t, idh_in),
                           (dv_t, dv_in), (dv01_t, dv01_in), (ixa_t, ixa_in),
                           (ixb_t, ixb_in), (dstf_t, dstf_in)):
                nc.sync.dma_start(out=t_[:], in_=p_[:])

            # ---- collective buffers ----
            agh = dpool.tile([Nc, F1], f16)
            hs_full = dpool.tile([NP_, F1], f16, addr_space="Shared")
            agz = dpool.tile([Nc, Fz], f16)
            z_full = dpool.tile([NP_, Fz], f16, addr_space="Shared")

            # ---- one GCN layer ----
            def layer(tabA, tabB, w_t, Fout, sh_t, store_cb, tagp):
                offA = offB = 0
                for b, K in enumerate(batches):
                    w0 = b * GK
                    bA, bB = batA[b], batB[b]
                    msgA = mpool.tile([P, mA_max, F1], f16, tag=tagp + "mA")
                    msgB = mpool.tile([P, mB_max, F1], f16, tag=tagp + "mB")
                    nc.gpsimd.dma_gather(
                        out_ap=msgA[:, :bA, :], in_ap=tabA,
                        idxs_ap=ixa_t[:, offA * 8:(offA + bA) * 8],
                        num_idxs=bA * P, num_idxs_reg=bA * P,
                        elem_size=F1, single_packet=False)
                    nc.gpsimd.dma_gather(
                        out_ap=msgB[:, :bB, :], in_ap=tabB,
                        idxs_ap=ixb_t[:, offB * 8:(offB + bB) * 8],
                        num_idxs=bB * P, num_idxs_reg=bB * P,
                        elem_size=F1, single_packet=False)
                    ca0 = cb0 = 0
                    for kw in range(K):
                        w = w0 + kw
                        CAi, CBi = int(CAw[w]), int(CBw[w])
                        CTi = CAi + CBi
                        d0 = int(doff[w])
                        S = pool.tile([P, CTmax, P], f16, tag=tagp + "S")
                        nc.vector.tensor_tensor(
                            out=S[:, :CTi, :],
                            in0=dstf_t[:, d0:d0 + CTi]
                                .rearrange("p (c o) -> p c o", o=1)
                                .to_broadcast([P, CTi, P]),
                            in1=iota_t[:, :CTi * P]
                                .rearrange("p (c m) -> p c m", m=P),
                            op=OP.is_equal)
                        ps = psA.tile([P, F1], f32, tag="agg")
                        for c in range(CAi):
                            nc.tensor.matmul(ps[:], S[:, c, :],
                                             msgA[:, ca0 + c, :],
                                             start=(c == 0), stop=False)
                        for c in range(CBi):
                            nc.tensor.matmul(ps[:], S[:, CAi + c, :],
                                             msgB[:, cb0 + c, :],
                                             start=False, stop=(c == CBi - 1))
                        ca0 += CAi
                        cb0 += CBi
                        t1 = pool.tile([P, P], f16, tag=tagp + "t1")
                        nc.scalar.mul(t1[:], ps[:], dv_t[:, w:w + 1])
                        psT = psA.tile([P, P], f16, tag="T")
                        nc.tensor.transpose(psT[:], t1[:], idh_t[:])
                        tT = pool.tile([P, P], f16, tag=tagp + "tT")
                        nc.vector.tensor_copy(tT[:], psT[:])
                        ps2 = psA.tile([P, Fout], f32, tag="mm2")
                        nc.tensor.matmul(ps2[:], tT[:], w_t[:],
                                         start=True, stop=True)
                        v = pool.tile([P, Fout], f32, tag=tagp + "v")
                        nc.vector.tensor_tensor(out=v[:], in0=ps2[:],
                                                in1=sh_t[:], op=OP.add)
                        store_cb(w, v)
                    offA += bA
                    offB += bB

            def store_h(w, v):
                u = pool.tile([P, F1], f32, tag="hu")
                nc.scalar.mul(u[:], v[:], dv01_t[:, w:w + 1])
                g = pool.tile([P, F1], f32, tag="hg")
                nc.scalar.mul(g[:], v[:], dv_t[:, w:w + 1])
                hs = pool.tile([P, F1], f16, tag="hs")
                nc.vector.tensor_tensor(out=hs[:], in0=g[:], in1=u[:], op=OP.max)
                nc.sync.dma_start(out=agh[w * P:(w + 1) * P, :], in_=hs[:])

            def store_z(w, v):
                u = pool.tile([P, F2], f32, tag="zu")
                nc.scalar.mul(u[:], v[:], 0.1)
                z = pool.tile([P, Fz], f16, tag="zst")
                nc.vector.memset(z[:, F2:], 0.0)
                nc.vector.tensor_tensor(out=z[:, :F2], in0=v[:], in1=u[:],
                                        op=OP.max)
                nc.sync.dma_start(out=agz[w * P:(w + 1) * P, :], in_=z[:])

            layer(xs_in[0:HALF, :], xs_in[HALF:NP_, :], w1_t, F1, sh1_t,
                  store_h, "a")
            nc.gpsimd.collective_compute(
                "AllGather", mybir.AluOpType.bypass,
                ins=[agh.opt()], outs=[hs_full.opt()], replica_groups=rg)

            layer(hs_full[0:HALF, :], hs_full[HALF:NP_, :], w2_t, F2, sh2_t,
                  store_z, "b")
            nc.gpsimd.collective_compute(
                "AllGather", mybir.AluOpType.bypass,
                ins=[agz.opt()], outs=[z_full.opt()], replica_groups=rg)

            # ---- decode: classes bound statically to z halves ----
            stage_ss = cpool.tile([P, DGT * 16], f32)
            stage_mj = cpool.tile([P, DGT * 16], f32)
            zA = z_full[0:HALF, :]
            zB = z_full[HALF:NP_, :]
            Fp = F2 - 1

            def cls_of(g):
                for k in range(4):
                    if cls_bases[k] <= g < cls_bases[k + 1]:
                        return k
                return 3

            for j in range(DGT // DB):
                g0 = j * DB
                k = cls_of(g0)             # DB groups per batch share a class
                assert cls_of(g0 + DB - 1) == k
                zzS = dpool2.tile([P, DB * 16, Fz], f16, tag="zzS")
                zzD = dpool2.tile([P, DB * 16, Fz], f16, tag="zzD")
                si_t = pool.tile([P, DB * 128], i16, tag="dsi")
                di_t = pool.tile([P, DB * 128], i16, tag="ddi")
                nc.sync.dma_start(out=si_t[:],
                                  in_=dsrc_in[:, g0 * 128:(g0 + DB) * 128])
                nc.sync.dma_start(out=di_t[:],
                                  in_=ddst_in[:, g0 * 128:(g0 + DB) * 128])
                nc.gpsimd.dma_gather(
                    out_ap=zzS[:, :, :], in_ap=(zB if k >= 2 else zA),
                    idxs_ap=si_t[:], num_idxs=DB * 2048,
                    num_idxs_reg=DB * 2048, elem_size=Fz,
                    single_packet=False)
                nc.gpsimd.dma_gather(
                    out_ap=zzD[:, :, :], in_ap=(zB if k % 2 else zA),
                    idxs_ap=di_t[:], num_idxs=DB * 2048,
                    num_idxs_reg=DB * 2048, elem_size=Fz,
                    single_packet=False)
                df = dpool2.tile([P, DB * 16, Fp], f16, tag="ddf")
                nc.vector.tensor_tensor(out=df[:], in0=zzS[:, :, 0:Fp],
                                        in1=zzD[:, :, 0:Fp], op=OP.subtract)
                sq = dpool2.tile([P, DB * 16, Fp], f16, tag="dsq")
                nc.vector.tensor_tensor(out=sq[:], in0=df[:], in1=df[:],
                                        op=OP.mult)
                nc.vector.reduce_sum(
                    out=stage_ss[:, g0 * 16:(g0 + DB) * 16]
                        .rearrange("p (c o) -> p c o", o=1),
                    in_=sq[:], axis=mybir.AxisListType.X)
                nc.vector.tensor_copy(stage_mj[:, g0 * 16:(g0 + DB) * 16],
                                      zzD[:, :, Fp])
            st_d = cpool.tile([P, DGT * 16], f32)
            nc.scalar.sqrt(st_d[:], stage_ss[:])
            st_v = cpool.tile([P, DGT * 16], f32)
            nc.vector.tensor_tensor(out=st_v[:], in0=stage_mj[:], in1=st_d[:],
                                    op=OP.subtract)
            st_o = cpool.tile([P, DGT * 16], f32)
            nc.scalar.activation(st_o[:], st_v[:], AF.Sigmoid)
            nc.sync.dma_start(out=out_dram[:], in_=st_o[:])
    nc.compile()
    return nc


# --------------------------------------------------------------------------
# public entry
# --------------------------------------------------------------------------
def _prep_inputs(x, edge_index, W1, b1, gamma1, beta1, mean1, var1,
                 W2, b2, gamma2, beta2, mean2, var2, n_cores):
    x = np.asarray(x, np.float32)
    edge_index = np.asarray(edge_index)
    ht = _build_host_tables(x, edge_index, n_cores)
    NP_, NWc, CTmax, DGT = (ht[k] for k in ("NP", "NWc", "CTmax", "DGT"))
    F1 = W1.shape[1]
    F2 = W2.shape[1]

    scale1 = np.asarray(gamma1) / np.sqrt(np.asarray(var1) + EPS)
    shift1 = (np.asarray(beta1) + (np.asarray(b1) - np.asarray(mean1)) * scale1).astype(np.float32)
    W1p = (np.asarray(W1) * scale1[None, :]).astype(np.float16)
    scale2 = np.asarray(gamma2) / np.sqrt(np.asarray(var2) + EPS)
    shift2 = (np.asarray(beta2) + (np.asarray(b2) - np.asarray(mean2)) * scale2).astype(np.float32)
    W2p = (np.asarray(W2) * scale2[None, :]).astype(np.float16)

    xs = np.zeros((NP_, F1), np.float32)
    xs[: ht["N"]] = x
    xs *= ht["dinv"][:, None]
    xs16 = xs.astype(np.float16)

    iota = np.tile(np.arange(P, dtype=np.float16)[None, :], (1, CTmax))
    iota = np.broadcast_to(iota, (P, CTmax * P)).copy()
    identh = np.eye(P, dtype=np.float16)
    sh1_rep = np.broadcast_to(shift1[None, :], (P, F1)).copy()
    sh2_rep = np.broadcast_to(shift2[None, :], (P, F2)).copy()

    in_maps = []
    for c in range(n_cores):
        in_maps.append({
            "xs": xs16, "w1": W1p, "w2": W2p,
            "shift1": sh1_rep, "shift2": sh2_rep,
            "iota": iota, "identh": identh,
            "dv": ht["dv_core"][c],
            "dv01": np.ascontiguousarray(0.1 * ht["dv_core"][c]),
            "idxA": ht["idxA_core"][c], "idxB": ht["idxB_core"][c],
            "dstf": ht["dstf_core"][c],
            "dsrc": ht["dsrc_core"][c], "ddst": ht["ddst_core"][c],
        })
    dims = dict(NP=NP_, NWc=NWc, CAw=ht["CAw"], CBw=ht["CBw"],
                CTmax=ht["CTmax"], doff=ht["doff"], batA=ht["batA"],
                batB=ht["batB"], F1=F1, F2=F2, DGT=DGT,
                batches=ht["batches"], cls_bases=ht["cls_bases"])
    return ht, dims, in_maps


def _assemble_output(ht, results, n_cores):
    E, DGT = ht["E"], ht["DGT"]
    out = np.empty(E, np.float32)
    EC = ht["EC"]
    for c in range(n_cores):
        e0 = c * EC
        arr = results[c]["out"]                         # [P, DGT*16]
        flat = arr.reshape(P, DGT, 16).transpose(1, 2, 0).reshape(-1)
        for base, ek in ht["perm_core"][c]:
            out[e0 + ek] = flat[base:base + len(ek)]
    return out


def kernel(x, edge_index, W1, b1, gamma1, beta1, mean1, var1,
           W2, b2, gamma2, beta2, mean2, var2, n_cores=8, _trace=False):
    from concourse.bass_utils import run_bass_kernel_spmd

    ht, dims, in_maps = _prep_inputs(
        x, edge_index, W1, b1, gamma1, beta1, mean1, var1,
        W2, b2, gamma2, beta2, mean2, var2, n_cores)
    nc = _build_program(dims["NP"], dims["NWc"], dims["CAw"], dims["CBw"],
                        dims["CTmax"], dims["doff"], dims["batA"],
                        dims["batB"], dims["F1"], dims["F2"], dims["DGT"],
                        dims["batches"], dims["cls_bases"], n_cores)
    try:
        res = run_bass_kernel_spmd(nc, in_maps, list(range(n_cores)), trace=_trace)
    except ModuleNotFoundError:
        res = run_bass_kernel_spmd(nc, in_maps, list(range(n_cores)), trace=False)
    kernel._last_results = res
    kernel._last_nc = nc
    return _assemble_output(ht, res.results, n_cores)


# revision 3
# speedup vs baseline: 1.0153x; 1.0153x over previous
"""GravityAE GNN message-passing kernel for 8 TRN2 NeuronCores (Bass/Tile), v3.

Math (GCN autoencoder, eval):
  scale_k = gamma_k/sqrt(var_k+eps); shift_k = beta_k + (b_k-mean_k)*scale_k
  Wkp = W_k*scale_k;  dinv[n] = 1/sqrt(in_deg incl self loop)
  xs = dinv*x  (fp16 DRAM table);  agg1[d] = sum_{e:dst=d} xs[src]
  h  = leaky(dinv_d*agg1 @ W1p + shift1);  hs = dinv*h  (fp16 table)
  z  = leaky(dinv_d*(sum hs[src]) @ W2p + shift2)
  out[e] = sigmoid(z[dst,64] - ||z[src,:64]-z[dst,:64]||)

Distribution: dst-sharded aggregation (49 windows of 128 nodes per core;
edges+self-loops sorted by dst, then split per window into two compacted
streams by src half since dma_gather indices are int16). Gathers use the
Pool-engine dma_gather ucode (one call per 4-window batch per half,
single_packet=False) which amortizes the ~1us SWDGE fixed cost over
thousands of descriptors. Segment-sum = S^T @ msg in PSUM with S built
by one fp16 is_equal per window; dense W matmul on the PE-transposed
window; leaky-relu as max(dinv*v, 0.1*dinv*v) with the scales applied by
the Act engine. AllGather (x8) rebuilds the hs and z tables between
stages; z is stored padded to 128 cols so decode rows are 256B. Decode
is edge-sharded, edges classed by (src-half, dst-half) so each 2-group
batch needs two dma_gathers; the host inverse-permutes the output.
"""
import numpy as np

P = 128
EPS = 1e-5
HALF = 32768          # int16 index limit for dma_gather
GK = 3                # windows per layer gather batch
DB = 2                # decode groups per gather batch


# --------------------------------------------------------------------------
# host-side preprocessing
# --------------------------------------------------------------------------
def _idx16(idx_flat, cols):
    """int16 index tile [128, cols]: slot i -> [i%16, i//16], replicated x8."""
    t = np.zeros((16, cols), np.int16)
    n = len(idx_flat)
    t[np.arange(n) % 16, np.arange(n) // 16] = idx_flat.astype(np.int16)
    return np.tile(t, (8, 1))


def _build_host_tables(x, edge_index, n_cores):
    N = x.shape[0]
    E = edge_index.shape[1]
    NW = ((N + P - 1) // P + n_cores - 1) // n_cores * n_cores
    NP_ = NW * P
    src = edge_index[0].astype(np.int64)
    dst = edge_index[1].astype(np.int64)
    s_all = np.concatenate([src, np.arange(N)])
    d_all = np.concatenate([dst, np.arange(N)])
    deg = np.bincount(d_all, minlength=NP_).astype(np.float64)
    dinv = np.zeros(NP_, np.float32)
    nz = deg > 0
    dinv[nz] = (1.0 / np.sqrt(deg[nz])).astype(np.float32)

    # sort by (dst window, src-half): gives per-window contiguous A then B runs
    win = d_all // P
    half = (s_all >= HALF).astype(np.int64)
    order = np.lexsort((half, d_all))
    s_sorted = s_all[order]
    d_sorted = d_all[order]
    h_sorted = half[order]

    cntAB = np.bincount(win[order] * 2 + h_sorted, minlength=NW * 2).reshape(NW, 2)
    NWc = NW // n_cores
    cpc = cntAB.reshape(n_cores, NWc, 2)
    CAw = np.maximum(1, np.ceil(cpc[:, :, 0].max(0) / P).astype(np.int64))  # [NWc]
    CBw = np.maximum(1, np.ceil(cpc[:, :, 1].max(0) / P).astype(np.int64))
    CTw = CAw + CBw
    CTmax = int(CTw.max())
    doff = np.zeros(NWc + 1, np.int64)
    np.cumsum(CTw, out=doff[1:])                      # dstf col offsets

    # per-window slot streams (per core, window-slot i): A then B
    starts = np.zeros(NW + 1, np.int64)
    np.cumsum(cntAB.sum(1), out=starts[1:])

    nbat = (NWc + GK - 1) // GK
    batches = [min(GK, NWc - b * GK) for b in range(nbat)]
    batA = [int(CAw[b * GK:b * GK + k].sum()) for b, k in enumerate(batches)]
    batB = [int(CBw[b * GK:b * GK + k].sum()) for b, k in enumerate(batches)]

    idxA_core, idxB_core, dstf_core, dv_core = [], [], [], []
    dv = dinv.reshape(NW, P).T                        # [P, NW]
    for c in range(n_cores):
        segsA, segsB = [], []
        dstf_c = np.full((int(doff[-1]), P), -1.0, np.float16)  # [cols, P]
        for b, k in enumerate(batches):
            ia = np.zeros(batA[b] * P, np.int64)
            ib = np.zeros(batB[b] * P, np.int64)
            oa = ob = 0
            for i in range(b * GK, b * GK + k):
                W = c * NWc + i
                sl = slice(starts[W], starts[W + 1])
                ss = s_sorted[sl]
                dd = d_sorted[sl] - W * P
                hh = h_sorted[sl]
                a, bb = ss[hh == 0], ss[hh == 1] - HALF
                ia[oa:oa + len(a)] = a
                ib[ob:ob + len(bb)] = bb
                da, db_ = dd[hh == 0], dd[hh == 1]
                col0 = doff[i]
                dstf_c.reshape(-1)[col0 * P:col0 * P + len(da)] = da.astype(np.float16)
                colB = (doff[i] + CAw[i]) * P
                dstf_c.reshape(-1)[colB:colB + len(db_)] = db_.astype(np.float16)
                oa += CAw[i] * P
                ob += CBw[i] * P
            segsA.append(_idx16(ia, batA[b] * 8))
            segsB.append(_idx16(ib, batB[b] * 8))
        idxA_core.append(np.concatenate(segsA, 1))
        idxB_core.append(np.concatenate(segsB, 1))
        dstf_core.append(np.ascontiguousarray(dstf_c.T))   # [P, sum(CTw)]
        dv_core.append(np.ascontiguousarray(dv[:, c * NWc:(c + 1) * NWc]))

    # ---- decode: 4 classes by (src-half, dst-half), groups of 2048 ----
    EC = (E + n_cores - 1) // n_cores
    cls_all = (src >= HALF) * 2 + (dst >= HALF)
    gcls = np.zeros((n_cores, 4), np.int64)
    per_core = []
    for c in range(n_cores):
        e0, e1 = c * EC, min((c + 1) * EC, E)
        ids = [np.nonzero(cls_all[e0:e1] == k)[0] for k in range(4)]
        gcls[c] = [(len(i) + 2047) // 2048 for i in ids]
        per_core.append((e0, ids))
    gmax = gcls.max(0)                                # uniform group counts
    gmax = np.maximum(gmax, 1)
    # pad each class to a DB multiple so every DB-batch is single-class
    gmax = (gmax + DB - 1) // DB * DB
    DGT = int(gmax.sum())
    g0 = np.zeros(5, np.int64)
    np.cumsum(gmax, out=g0[1:])

    dsrc_core, ddst_core, perm_core = [], [], []
    for c in range(n_cores):
        e0, ids = per_core[c]
        sflat = np.zeros(DGT * 2048, np.int64)
        dflat = np.zeros(DGT * 2048, np.int64)
        perm = []                                     # (slot, local_edge_id)
        for k in range(4):
            base = g0[k] * 2048
            ek = ids[k]
            sk = src[e0 + ek]
            dk = dst[e0 + ek]
            sflat[base:base + len(ek)] = sk - (HALF if k >= 2 else 0)
            dflat[base:base + len(ek)] = dk - (HALF if k % 2 else 0)
            perm.append((base, ek))
        dsrc_core.append(_idx16(sflat, DGT * 128))
        ddst_core.append(_idx16(dflat, DGT * 128))
        perm_core.append(perm)

    return dict(N=N, E=E, NW=NW, NP=NP_, CAw=CAw, CBw=CBw, CTmax=CTmax,
                doff=doff, batA=batA, batB=batB, NWc=NWc,
                EC=EC, DGT=DGT, batches=batches, cls_bases=g0,
                dinv=dinv, idxA_core=idxA_core, idxB_core=idxB_core,
                dstf_core=dstf_core, dv_core=dv_core,
                dsrc_core=dsrc_core, ddst_core=ddst_core,
                perm_core=perm_core)


# --------------------------------------------------------------------------
# bass program
# --------------------------------------------------------------------------
def _build_program(NP_, NWc, CAw, CBw, CTmax, doff, batA, batB, F1, F2, DGT,
                   batches, cls_bases, n_cores):
    import concourse.bass as bass
    import concourse.tile as tile
    from concourse import bacc, mybir

    dt = mybir.dt
    f32 = dt.float32
    f16 = dt.float16
    i16 = dt.int16
    Nc = NWc * P
    SA, SB, ST = sum(batA), sum(batB), int(doff[-1])
    mA_max, mB_max = max(batA), max(batB)
    Fz = 128                      # z stored padded to 128 cols (256B rows)
    NB = NP_ - HALF               # rows in half B

    nc = bacc.Bacc("TRN2", target_bir_lowering=False, debug=False,
                   num_devices=n_cores)
    xs_in = nc.declare_dram_parameter("xs", [NP_, F1], f16, isOutput=False)
    w1_in = nc.declare_dram_parameter("w1", [F1, F1], f16, isOutput=False)
    w2_in = nc.declare_dram_parameter("w2", [F1, F2], f16, isOutput=False)
    sh1_in = nc.declare_dram_parameter("shift1", [P, F1], f32, isOutput=False)
    sh2_in = nc.declare_dram_parameter("shift2", [P, F2], f32, isOutput=False)
    iota_in = nc.declare_dram_parameter("iota", [P, CTmax * P], f16, isOutput=False)
    idh_in = nc.declare_dram_parameter("identh", [P, P], f16, isOutput=False)
    dv_in = nc.declare_dram_parameter("dv", [P, NWc], f32, isOutput=False)
    dv01_in = nc.declare_dram_parameter("dv01", [P, NWc], f32, isOutput=False)
    ixa_in = nc.declare_dram_parameter("idxA", [P, SA * 8], i16, isOutput=False)
    ixb_in = nc.declare_dram_parameter("idxB", [P, SB * 8], i16, isOutput=False)
    dstf_in = nc.declare_dram_parameter("dstf", [P, ST], f16, isOutput=False)
    dsrc_in = nc.declare_dram_parameter("dsrc", [P, DGT * 128], i16, isOutput=False)
    ddst_in = nc.declare_dram_parameter("ddst", [P, DGT * 128], i16, isOutput=False)
    out_dram = nc.declare_dram_parameter("out", [P, DGT * 16], f32, isOutput=True)

    rg = [list(range(n_cores))]
    AF = mybir.ActivationFunctionType
    OP = mybir.AluOpType

    with tile.TileContext(nc) as tc:
        with (
            tc.tile_pool(name="const", bufs=1) as cpool,
            tc.tile_pool(name="msgp", bufs=2) as mpool,
            tc.tile_pool(name="sbuf", bufs=3) as pool,
            tc.tile_pool(name="dec", bufs=2) as dpool2,
            tc.tile_pool(name="psA", bufs=2, space="PSUM") as psA,
            tc.tile_pool(name="dram", bufs=1, space="DRAM") as dpool,
        ):
            # ---- constants ----
            w1_t = cpool.tile([F1, F1], f16)
            w2_t = cpool.tile([F1, F2], f16)
            sh1_t = cpool.tile([P, F1], f32)
            sh2_t = cpool.tile([P, F2], f32)
            iota_t = cpool.tile([P, CTmax * P], f16)
            idh_t = cpool.tile([P, P], f16)
            dv_t = cpool.tile([P, NWc], f32)
            dv01_t = cpool.tile([P, NWc], f32)
            ixa_t = cpool.tile([P, SA * 8], i16)
            ixb_t = cpool.tile([P, SB * 8], i16)
            dstf_t = cpool.tile([P, ST], f16)
            for t_, p_ in ((w1_t, w1_in), (w2_t, w2_in), (sh1_t, sh1_in),
                           (sh2_t, sh2_in), (iota_t, iota_in), (idh_t, idh_in),
                           (dv_t, dv_in), (dv01_t, dv01_in), (ixa_t, ixa_in),
                           (ixb_t, ixb_in), (dstf_t, dstf_in)):
                nc.sync.dma_start(out=t_[:], in_=p_[:])

            # ---- collective buffers ----
            agh = dpool.tile([Nc, F1], f16)
            hs_full = dpool.tile([NP_, F1], f16, addr_space="Shared")
            agz = dpool.tile([Nc, Fz], f16)
            z_full = dpool.tile([NP_, Fz], f16, addr_space="Shared")

            # ---- one GCN layer ----
            def layer(tabA, tabB, w_t, Fout, sh_t, store_cb, tagp):
                offA = offB = 0
                for b, K in enumerate(batches):
                    w0 = b * GK
                    bA, bB = batA[b], batB[b]
                    msgA = mpool.tile([P, mA_max, F1], f16, tag=tagp + "mA")
                    msgB = mpool.tile([P, mB_max, F1], f16, tag=tagp + "mB")
                    nc.gpsimd.dma_gather(
                        out_ap=msgA[:, :bA, :], in_ap=tabA,
                        idxs_ap=ixa_t[:, offA * 8:(offA + bA) * 8],
                        num_idxs=bA * P, num_idxs_reg=bA * P,
                        elem_size=F1, single_packet=False)
                    nc.gpsimd.dma_gather(
                        out_ap=msgB[:, :bB, :], in_ap=tabB,
                        idxs_ap=ixb_t[:, offB * 8:(offB + bB) * 8],
                        num_idxs=bB * P, num_idxs_reg=bB * P,
                        elem_size=F1, single_packet=False)
                    ca0 = cb0 = 0
                    for kw in range(K):
                        w = w0 + kw
                        CAi, CBi = int(CAw[w]), int(CBw[w])
                        CTi = CAi + CBi
                        d0 = int(doff[w])
                        S = pool.tile([P, CTmax, P], f16, tag=tagp + "S")
                        nc.vector.tensor_tensor(
                            out=S[:, :CTi, :],
                            in0=dstf_t[:, d0:d0 + CTi]
                                .rearrange("p (c o) -> p c o", o=1)
                                .to_broadcast([P, CTi, P]),
                            in1=iota_t[:, :CTi * P]
                                .rearrange("p (c m) -> p c m", m=P),
                            op=OP.is_equal)
                        ps = psA.tile([P, F1], f32, tag="agg")
                        for c in range(CAi):
                            nc.tensor.matmul(ps[:], S[:, c, :],
                                             msgA[:, ca0 + c, :],
                                             start=(c == 0), stop=False)
                        for c in range(CBi):
                            nc.tensor.matmul(ps[:], S[:, CAi + c, :],
                                             msgB[:, cb0 + c, :],
                                             start=False, stop=(c == CBi - 1))
                        ca0 += CAi
                        cb0 += CBi
                        t1 = pool.tile([P, P], f16, tag=tagp + "t1")
                        nc.scalar.mul(t1[:], ps[:], dv_t[:, w:w + 1])
                        psT = psA.tile([P, P], f16, tag="T")
                        nc.tensor.transpose(psT[:], t1[:], idh_t[:])
                        tT = pool.tile([P, P], f16, tag=tagp + "tT")
                        nc.vector.tensor_copy(tT[:], psT[:])
                        ps2 = psA.tile([P, Fout], f32, tag="mm2")
                        nc.tensor.matmul(ps2[:], tT[:], w_t[:],
                                         start=True, stop=True)
                        v = pool.tile([P, Fout], f32, tag=tagp + "v")
                        nc.vector.tensor_tensor(out=v[:], in0=ps2[:],
                                                in1=sh_t[:], op=OP.add)
                        store_cb(w, v)
                    offA += bA
                    offB += bB

            def store_h(w, v):
                u = pool.tile([P, F1], f32, tag="hu")
                nc.scalar.mul(u[:], v[:], dv01_t[:, w:w + 1])
                g = pool.tile([P, F1], f32, tag="hg")
                nc.scalar.mul(g[:], v[:], dv_t[:, w:w + 1])
                hs = pool.tile([P, F1], f16, tag="hs")
                nc.vector.tensor_tensor(out=hs[:], in0=g[:], in1=u[:], op=OP.max)
                nc.sync.dma_start(out=agh[w * P:(w + 1) * P, :], in_=hs[:])

            def store_z(w, v):
                u = pool.tile([P, F2], f32, tag="zu")
                nc.scalar.mul(u[:], v[:], 0.1)
                z = pool.tile([P, Fz], f16, tag="zst")
                nc.vector.memset(z[:, F2:], 0.0)
                nc.vector.tensor_tensor(out=z[:, :F2], in0=v[:], in1=u[:],
                                        op=OP.max)
                nc.sync.dma_start(out=agz[w * P:(w + 1) * P, :], in_=z[:])

            layer(xs_in[0:HALF, :], xs_in[HALF:NP_, :], w1_t, F1, sh1_t,
                  store_h, "a")
            nc.gpsimd.collective_compute(
                "AllGather", mybir.AluOpType.bypass,
                ins=[agh.opt()], outs=[hs_full.opt()], replica_groups=rg)

            layer(hs_full[0:HALF, :], hs_full[HALF:NP_, :], w2_t, F2, sh2_t,
                  store_z, "b")
            nc.gpsimd.collective_compute(
                "AllGather", mybir.AluOpType.bypass,
                ins=[agz.opt()], outs=[z_full.opt()], replica_groups=rg)

            # ---- decode: classes bound statically to z halves ----
            stage_ss = cpool.tile([P, DGT * 16], f32)
            stage_mj = cpool.tile([P, DGT * 16], f32)
            zA = z_full[0:HALF, :]
            zB = z_full[HALF:NP_, :]
            Fp = F2 - 1

            def cls_of(g):
                for k in range(4):
                    if cls_bases[k] <= g < cls_bases[k + 1]:
                        return k
                return 3

            for j in range(DGT // DB):
                g0 = j * DB
                k = cls_of(g0)             # DB groups per batch share a class
                assert cls_of(g0 + DB - 1) == k
                zzS = dpool2.tile([P, DB * 16, Fz], f16, tag="zzS")
                zzD = dpool2.tile([P, DB * 16, Fz], f16, tag="zzD")
                si_t = pool.tile([P, DB * 128], i16, tag="dsi")
                di_t = pool.tile([P, DB * 128], i16, tag="ddi")
                nc.sync.dma_start(out=si_t[:],
                                  in_=dsrc_in[:, g0 * 128:(g0 + DB) * 128])
                nc.sync.dma_start(out=di_t[:],
                                  in_=ddst_in[:, g0 * 128:(g0 + DB) * 128])
                nc.gpsimd.dma_gather(
                    out_ap=zzS[:, :, :], in_ap=(zB if k >= 2 else zA),
                    idxs_ap=si_t[:], num_idxs=DB * 2048,
                    num_idxs_reg=DB * 2048, elem_size=Fz,
                    single_packet=False)
                nc.gpsimd.dma_gather(
                    out_ap=zzD[:, :, :], in_ap=(zB if k % 2 else zA),
                    idxs_ap=di_t[:], num_idxs=DB * 2048,
                    num_idxs_reg=DB * 2048, elem_size=Fz,
                    single_packet=False)
                df = dpool2.tile([P, DB * 16, Fp], f16, tag="ddf")
                nc.vector.tensor_tensor(out=df[:], in0=zzS[:, :, 0:Fp],
                                        in1=zzD[:, :, 0:Fp], op=OP.subtract)
                sq = dpool2.tile([P, DB * 16, Fp], f16, tag="dsq")
                nc.vector.tensor_tensor(out=sq[:], in0=df[:], in1=df[:],
                                        op=OP.mult)
                nc.vector.reduce_sum(
                    out=stage_ss[:, g0 * 16:(g0 + DB) * 16]
                        .rearrange("p (c o) -> p c o", o=1),
                    in_=sq[:], axis=mybir.AxisListType.X)
                nc.vector.tensor_copy(stage_mj[:, g0 * 16:(g0 + DB) * 16],
                                      zzD[:, :, Fp])
            st_d = cpool.tile([P, DGT * 16], f32)
            nc.scalar.sqrt(st_d[:], stage_ss[:])
            st_v = cpool.tile([P, DGT * 16], f32)
            nc.vector.tensor_tensor(out=st_v[:], in0=stage_mj[:], in1=st_d[:],
                                    op=OP.subtract)
            st_o = cpool.tile([P, DGT * 16], f32)
            nc.scalar.activation(st_o[:], st_v[:], AF.Sigmoid)
            nc.sync.dma_start(out=out_dram[:], in_=st_o[:])
    nc.compile()
    return nc


# --------------------------------------------------------------------------
# public entry
# --------------------------------------------------------------------------
def _prep_inputs(x, edge_index, W1, b1, gamma1, beta1, mean1, var1,
                 W2, b2, gamma2, beta2, mean2, var2, n_cores):
    x = np.asarray(x, np.float32)
    edge_index = np.asarray(edge_index)
    ht = _build_host_tables(x, edge_index, n_cores)
    NP_, NWc, CTmax, DGT = (ht[k] for k in ("NP", "NWc", "CTmax", "DGT"))
    F1 = W1.shape[1]
    F2 = W2.shape[1]

    scale1 = np.asarray(gamma1) / np.sqrt(np.asarray(var1) + EPS)
    shift1 = (np.asarray(beta1) + (np.asarray(b1) - np.asarray(mean1)) * scale1).astype(np.float32)
    W1p = (np.asarray(W1) * scale1[None, :]).astype(np.float16)
    scale2 = np.asarray(gamma2) / np.sqrt(np.asarray(var2) + EPS)
    shift2 = (np.asarray(beta2) + (np.asarray(b2) - np.asarray(mean2)) * scale2).astype(np.float32)
    W2p = (np.asarray(W2) * scale2[None, :]).astype(np.float16)

    xs = np.zeros((NP_, F1), np.float32)
    xs[: ht["N"]] = x
    xs *= ht["dinv"][:, None]
    xs16 = xs.astype(np.float16)

    iota = np.tile(np.arange(P, dtype=np.float16)[None, :], (1, CTmax))
    iota = np.broadcast_to(iota, (P, CTmax * P)).copy()
    identh = np.eye(P, dtype=np.float16)
    sh1_rep = np.broadcast_to(shift1[None, :], (P, F1)).copy()
    sh2_rep = np.broadcast_to(shift2[None, :], (P, F2)).copy()

    in_maps = []
    for c in range(n_cores):
        in_maps.append({
            "xs": xs16, "w1": W1p, "w2": W2p,
            "shift1": sh1_rep, "shift2": sh2_rep,
            "iota": iota, "identh": identh,
            "dv": ht["dv_core"][c],
            "dv01": np.ascontiguousarray(0.1 * ht["dv_core"][c]),
            "idxA": ht["idxA_core"][c], "idxB": ht["idxB_core"][c],
            "dstf": ht["dstf_core"][c],
            "dsrc": ht["dsrc_core"][c], "ddst": ht["ddst_core"][c],
        })
    dims = dict(NP=NP_, NWc=NWc, CAw=ht["CAw"], CBw=ht["CBw"],
                CTmax=ht["CTmax"], doff=ht["doff"], batA=ht["batA"],
                batB=ht["batB"], F1=F1, F2=F2, DGT=DGT,
                batches=ht["batches"], cls_bases=ht["cls_bases"])
    return ht, dims, in_maps


def _assemble_output(ht, results, n_cores):
    E, DGT = ht["E"], ht["DGT"]
    out = np.empty(E, np.float32)
    EC = ht["EC"]
    for c in range(n_cores):
        e0 = c * EC
        arr = results[c]["out"]                         # [P, DGT*16]
        flat = arr.reshape(P, DGT, 16).transpose(1, 2, 0).reshape(-1)
        for base, ek in ht["perm_core"][c]:
            out[e0 + ek] = flat[base:base + len(ek)]
    return out


_program_cache = {}


def _cached_program(dims, n_cores):
    key = (dims["NP"], dims["NWc"], tuple(dims["CAw"]), tuple(dims["CBw"]),
           dims["CTmax"], tuple(dims["doff"]), tuple(dims["batA"]),
           tuple(dims["batB"]), dims["F1"], dims["F2"], dims["DGT"],
           tuple(dims["batches"]), tuple(dims["cls_bases"]), n_cores)
    if key not in _program_cache:
        _program_cache[key] = _build_program(
            dims["NP"], dims["NWc"], dims["CAw"], dims["CBw"],
            dims["CTmax"], dims["doff"], dims["batA"],
            dims["batB"], dims["F1"], dims["F2"], dims["DGT"],
            dims["batches"], dims["cls_bases"], n_cores)
    return _program_cache[key]


def kernel(x, edge_index, W1, b1, gamma1, beta1, mean1, var1,
           W2, b2, gamma2, beta2, mean2, var2, n_cores=8, _trace=False):
    from concourse.bass_utils import run_bass_kernel_spmd

    ht, dims, in_maps = _prep_inputs(
        x, edge_index, W1, b1, gamma1, beta1, mean1, var1,
        W2, b2, gamma2, beta2, mean2, var2, n_cores)
    nc = _cached_program(dims, n_cores)
    try:
        res = run_bass_kernel_spmd(nc, in_maps, list(range(n_cores)), trace=_trace)
    except ModuleNotFoundError:
        res = run_bass_kernel_spmd(nc, in_maps, list(range(n_cores)), trace=False)
    kernel._last_results = res
    kernel._last_nc = nc
    return _assemble_output(ht, res.results, n_cores)


# revision 4
# speedup vs baseline: 1.0265x; 1.0110x over previous
"""GravityAE GNN message-passing kernel for 8 TRN2 NeuronCores (Bass/Tile), v3.

Math (GCN autoencoder, eval):
  scale_k = gamma_k/sqrt(var_k+eps); shift_k = beta_k + (b_k-mean_k)*scale_k
  Wkp = W_k*scale_k;  dinv[n] = 1/sqrt(in_deg incl self loop)
  xs = dinv*x  (fp16 DRAM table);  agg1[d] = sum_{e:dst=d} xs[src]
  h  = leaky(dinv_d*agg1 @ W1p + shift1);  hs = dinv*h  (fp16 table)
  z  = leaky(dinv_d*(sum hs[src]) @ W2p + shift2)
  out[e] = sigmoid(z[dst,64] - ||z[src,:64]-z[dst,:64]||)

Distribution: dst-sharded aggregation (49 windows of 128 nodes per core;
edges+self-loops sorted by dst, then split per window into two compacted
streams by src half since dma_gather indices are int16). Gathers use the
Pool-engine dma_gather ucode (one call per 4-window batch per half,
single_packet=False) which amortizes the ~1us SWDGE fixed cost over
thousands of descriptors. Segment-sum = S^T @ msg in PSUM with S built
by one fp16 is_equal per window; dense W matmul on the PE-transposed
window; leaky-relu as max(dinv*v, 0.1*dinv*v) with the scales applied by
the Act engine. AllGather (x8) rebuilds the hs and z tables between
stages; z is stored padded to 128 cols so decode rows are 256B. Decode
is edge-sharded, edges classed by (src-half, dst-half) so each 2-group
batch needs two dma_gathers; the host inverse-permutes the output.
"""
import numpy as np

P = 128
EPS = 1e-5
HALF = 32768          # int16 index limit for dma_gather
GK = 3                # windows per layer gather batch
DB = 2                # decode groups per gather batch


# --------------------------------------------------------------------------
# host-side preprocessing
# --------------------------------------------------------------------------
def _idx16(idx_flat, cols):
    """int16 index tile [128, cols]: slot i -> [i%16, i//16], replicated x8."""
    t = np.zeros((16, cols), np.int16)
    n = len(idx_flat)
    t[np.arange(n) % 16, np.arange(n) // 16] = idx_flat.astype(np.int16)
    return np.tile(t, (8, 1))


def _build_host_tables(x, edge_index, n_cores):
    N = x.shape[0]
    E = edge_index.shape[1]
    NW = ((N + P - 1) // P + n_cores - 1) // n_cores * n_cores
    NP_ = NW * P
    src = edge_index[0].astype(np.int64)
    dst = edge_index[1].astype(np.int64)
    s_all = np.concatenate([src, np.arange(N)])
    d_all = np.concatenate([dst, np.arange(N)])
    deg = np.bincount(d_all, minlength=NP_).astype(np.float64)
    dinv = np.zeros(NP_, np.float32)
    nz = deg > 0
    dinv[nz] = (1.0 / np.sqrt(deg[nz])).astype(np.float32)

    # sort by (dst window, src-half): gives per-window contiguous A then B runs
    win = d_all // P
    half = (s_all >= HALF).astype(np.int64)
    order = np.lexsort((half, d_all))
    s_sorted = s_all[order]
    d_sorted = d_all[order]
    h_sorted = half[order]

    cntAB = np.bincount(win[order] * 2 + h_sorted, minlength=NW * 2).reshape(NW, 2)
    NWc = NW // n_cores
    cpc = cntAB.reshape(n_cores, NWc, 2)
    CAw = np.maximum(1, np.ceil(cpc[:, :, 0].max(0) / P).astype(np.int64))  # [NWc]
    CBw = np.maximum(1, np.ceil(cpc[:, :, 1].max(0) / P).astype(np.int64))
    CTw = CAw + CBw
    CTmax = int(CTw.max())
    doff = np.zeros(NWc + 1, np.int64)
    np.cumsum(CTw, out=doff[1:])                      # dstf col offsets

    # per-window slot streams (per core, window-slot i): A then B
    starts = np.zeros(NW + 1, np.int64)
    np.cumsum(cntAB.sum(1), out=starts[1:])

    nbat = (NWc + GK - 1) // GK
    batches = [min(GK, NWc - b * GK) for b in range(nbat)]
    batA = [int(CAw[b * GK:b * GK + k].sum()) for b, k in enumerate(batches)]
    batB = [int(CBw[b * GK:b * GK + k].sum()) for b, k in enumerate(batches)]

    idxA_core, idxB_core, dstf_core, dv_core = [], [], [], []
    dv = dinv.reshape(NW, P).T                        # [P, NW]
    for c in range(n_cores):
        segsA, segsB = [], []
        dstf_c = np.full((int(doff[-1]), P), -1.0, np.float16)  # [cols, P]
        for b, k in enumerate(batches):
            ia = np.zeros(batA[b] * P, np.int64)
            ib = np.zeros(batB[b] * P, np.int64)
            oa = ob = 0
            for i in range(b * GK, b * GK + k):
                W = c * NWc + i
                sl = slice(starts[W], starts[W + 1])
                ss = s_sorted[sl]
                dd = d_sorted[sl] - W * P
                hh = h_sorted[sl]
                a, bb = ss[hh == 0], ss[hh == 1] - HALF
                ia[oa:oa + len(a)] = a
                ib[ob:ob + len(bb)] = bb
                da, db_ = dd[hh == 0], dd[hh == 1]
                col0 = doff[i]
                dstf_c.reshape(-1)[col0 * P:col0 * P + len(da)] = da.astype(np.float16)
                colB = (doff[i] + CAw[i]) * P
                dstf_c.reshape(-1)[colB:colB + len(db_)] = db_.astype(np.float16)
                oa += CAw[i] * P
                ob += CBw[i] * P
            segsA.append(_idx16(ia, batA[b] * 8))
            segsB.append(_idx16(ib, batB[b] * 8))
        idxA_core.append(np.concatenate(segsA, 1))
        idxB_core.append(np.concatenate(segsB, 1))
        dstf_core.append(np.ascontiguousarray(dstf_c.T))   # [P, sum(CTw)]
        dv_core.append(np.ascontiguousarray(dv[:, c * NWc:(c + 1) * NWc]))

    # ---- decode: 4 classes by (src-half, dst-half), groups of 2048 ----
    EC = (E + n_cores - 1) // n_cores
    cls_all = (src >= HALF) * 2 + (dst >= HALF)
    gcls = np.zeros((n_cores, 4), np.int64)
    per_core = []
    for c in range(n_cores):
        e0, e1 = c * EC, min((c + 1) * EC, E)
        ids = [np.nonzero(cls_all[e0:e1] == k)[0] for k in range(4)]
        gcls[c] = [(len(i) + 2047) // 2048 for i in ids]
        per_core.append((e0, ids))
    gmax = gcls.max(0)                                # uniform group counts
    gmax = np.maximum(gmax, 1)
    # pad each class to a DB multiple so every DB-batch is single-class
    gmax = (gmax + DB - 1) // DB * DB
    DGT = int(gmax.sum())
    g0 = np.zeros(5, np.int64)
    np.cumsum(gmax, out=g0[1:])

    dsrc_core, ddst_core, perm_core = [], [], []
    for c in range(n_cores):
        e0, ids = per_core[c]
        sflat = np.zeros(DGT * 2048, np.int64)
        dflat = np.zeros(DGT * 2048, np.int64)
        perm = []                                     # (slot, local_edge_id)
        for k in range(4):
            base = g0[k] * 2048
            ek = ids[k]
            sk = src[e0 + ek]
            dk = dst[e0 + ek]
            sflat[base:base + len(ek)] = sk - (HALF if k >= 2 else 0)
            dflat[base:base + len(ek)] = dk - (HALF if k % 2 else 0)
            perm.append((base, ek))
        dsrc_core.append(_idx16(sflat, DGT * 128))
        ddst_core.append(_idx16(dflat, DGT * 128))
        perm_core.append(perm)

    return dict(N=N, E=E, NW=NW, NP=NP_, CAw=CAw, CBw=CBw, CTmax=CTmax,
                doff=doff, batA=batA, batB=batB, NWc=NWc,
                EC=EC, DGT=DGT, batches=batches, cls_bases=g0,
                dinv=dinv, idxA_core=idxA_core, idxB_core=idxB_core,
                dstf_core=dstf_core, dv_core=dv_core,
                dsrc_core=dsrc_core, ddst_core=ddst_core,
                perm_core=perm_core)


def _slim_dma_gather(g, out_ap, in_ap, idxs_ap, num_idxs, elem_size, elem_step):
    """dma_gather with elem_size < row stride (stride must be a 256B multiple).

    Mirrors BassGpsimd.dma_gather's lowering but permits sub-256B elements,
    which the non-transpose ucode path handles (HW-verified)."""
    import concourse.mybir as mybir
    import concourse.ap_utils as ap_utils
    from concourse._compat import exact_div
    assert idxs_ap.dtype == mybir.dt.int16
    assert in_ap.dtype == out_ap.dtype
    assert ap_utils.ap_is_contiguous(out_ap.ap[1:])
    assert ap_utils.ap_is_contiguous(idxs_ap.ap[1:])
    assert in_ap.ap[0][0] == elem_step
    stride_bytes_256 = exact_div(elem_step * mybir.dt.size(in_ap.dtype), 256)
    _in_ap = g.lower_ap_dma(in_ap, for_custom_bir_dma=True)
    _idxs_ap = g.lower_ap(idxs_ap)
    _out_ap = g.lower_ap(out_ap)
    return g.add_instruction(
        mybir.InstDMAGatherAnt(
            name=g.bass.get_next_instruction_name(),
            ins=[*_in_ap, _idxs_ap, g.lower_val_access(g.to_reg(num_idxs))],
            outs=[_out_ap],
            transpose=False, num_idxs=num_idxs, elem_size=elem_size,
            stride_bytes_256=stride_bytes_256, gen_mode=0,
            single_packet=False, queue_num=0, sbuf_tokens_per_rank=0,
            sbuf_free_dim_per_rank=0, sbuf_free_dim_pad_per_rank=0,
            sbuf_byte_offset=0))


# --------------------------------------------------------------------------
# bass program
# --------------------------------------------------------------------------
def _build_program(NP_, NWc, CAw, CBw, CTmax, doff, batA, batB, F1, F2, DGT,
                   batches, cls_bases, n_cores):
    import concourse.bass as bass
    import concourse.tile as tile
    from concourse import bacc, mybir

    dt = mybir.dt
    f32 = dt.float32
    f16 = dt.float16
    i16 = dt.int16
    Nc = NWc * P
    SA, SB, ST = sum(batA), sum(batB), int(doff[-1])
    mA_max, mB_max = max(batA), max(batB)
    Fz = 128                      # z stored padded to 128 cols (256B rows)
    NB = NP_ - HALF               # rows in half B

    nc = bacc.Bacc("TRN2", target_bir_lowering=False, debug=False,
                   num_devices=n_cores)
    xs_in = nc.declare_dram_parameter("xs", [NP_, F1], f16, isOutput=False)
    w1_in = nc.declare_dram_parameter("w1", [F1, F1], f16, isOutput=False)
    w2_in = nc.declare_dram_parameter("w2", [F1, F2], f16, isOutput=False)
    sh1_in = nc.declare_dram_parameter("shift1", [P, F1], f32, isOutput=False)
    sh2_in = nc.declare_dram_parameter("shift2", [P, F2], f32, isOutput=False)
    iota_in = nc.declare_dram_parameter("iota", [P, CTmax * P], f16, isOutput=False)
    idh_in = nc.declare_dram_parameter("identh", [P, P], f16, isOutput=False)
    dv_in = nc.declare_dram_parameter("dv", [P, NWc], f32, isOutput=False)
    dv01_in = nc.declare_dram_parameter("dv01", [P, NWc], f32, isOutput=False)
    ixa_in = nc.declare_dram_parameter("idxA", [P, SA * 8], i16, isOutput=False)
    ixb_in = nc.declare_dram_parameter("idxB", [P, SB * 8], i16, isOutput=False)
    dstf_in = nc.declare_dram_parameter("dstf", [P, ST], f16, isOutput=False)
    dsrc_in = nc.declare_dram_parameter("dsrc", [P, DGT * 128], i16, isOutput=False)
    ddst_in = nc.declare_dram_parameter("ddst", [P, DGT * 128], i16, isOutput=False)
    out_dram = nc.declare_dram_parameter("out", [P, DGT * 16], f32, isOutput=True)

    rg = [list(range(n_cores))]
    AF = mybir.ActivationFunctionType
    OP = mybir.AluOpType

    with tile.TileContext(nc) as tc:
        with (
            tc.tile_pool(name="const", bufs=1) as cpool,
            tc.tile_pool(name="msgp", bufs=2) as mpool,
            tc.tile_pool(name="sbuf", bufs=4) as pool,
            tc.tile_pool(name="dec", bufs=2) as dpool2,
            tc.tile_pool(name="psA", bufs=4, space="PSUM") as psA,
            tc.tile_pool(name="psB", bufs=2, space="PSUM") as psB,
            tc.tile_pool(name="dram", bufs=1, space="DRAM") as dpool,
        ):
            # ---- constants ----
            w1_t = cpool.tile([F1, F1], f16)
            w2_t = cpool.tile([F1, F2], f16)
            sh1_t = cpool.tile([P, F1], f32)
            sh2_t = cpool.tile([P, F2], f32)
            iota_t = cpool.tile([P, CTmax * P], f16)
            idh_t = cpool.tile([P, P], f16)
            dv_t = cpool.tile([P, NWc], f32)
            dv01_t = cpool.tile([P, NWc], f32)
            ixa_t = cpool.tile([P, SA * 8], i16)
            ixb_t = cpool.tile([P, SB * 8], i16)
            dstf_t = cpool.tile([P, ST], f16)
            for t_, p_ in ((w1_t, w1_in), (w2_t, w2_in), (sh1_t, sh1_in),
                           (sh2_t, sh2_in), (iota_t, iota_in), (idh_t, idh_in),
                           (dv_t, dv_in), (dv01_t, dv01_in), (ixa_t, ixa_in),
                           (ixb_t, ixb_in), (dstf_t, dstf_in)):
                nc.sync.dma_start(out=t_[:], in_=p_[:])

            # ---- collective buffers ----
            agh = dpool.tile([Nc, F1], f16)
            hs_full = dpool.tile([NP_, F1], f16, addr_space="Shared")
            agz = dpool.tile([Nc, Fz], f16)
            z_full = dpool.tile([NP_, Fz], f16, addr_space="Shared")

            # ---- one GCN layer ----
            def layer(tabA, tabB, w_t, Fout, sh_t, store_cb, tagp):
                offA = offB = 0
                for b, K in enumerate(batches):
                    w0 = b * GK
                    bA, bB = batA[b], batB[b]
                    msgA = mpool.tile([P, mA_max, F1], f16, tag=tagp + "mA")
                    msgB = mpool.tile([P, mB_max, F1], f16, tag=tagp + "mB")
                    nc.gpsimd.dma_gather(
                        out_ap=msgA[:, :bA, :], in_ap=tabA,
                        idxs_ap=ixa_t[:, offA * 8:(offA + bA) * 8],
                        num_idxs=bA * P, num_idxs_reg=bA * P,
                        elem_size=F1, single_packet=False)
                    nc.gpsimd.dma_gather(
                        out_ap=msgB[:, :bB, :], in_ap=tabB,
                        idxs_ap=ixb_t[:, offB * 8:(offB + bB) * 8],
                        num_idxs=bB * P, num_idxs_reg=bB * P,
                        elem_size=F1, single_packet=False)
                    ca0 = cb0 = 0
                    for kw in range(K):
                        w = w0 + kw
                        CAi, CBi = int(CAw[w]), int(CBw[w])
                        CTi = CAi + CBi
                        d0 = int(doff[w])
                        S = pool.tile([P, CTmax, P], f16, tag=tagp + "S")
                        nc.vector.tensor_tensor(
                            out=S[:, :CTi, :],
                            in0=dstf_t[:, d0:d0 + CTi]
                                .rearrange("p (c o) -> p c o", o=1)
                                .to_broadcast([P, CTi, P]),
                            in1=iota_t[:, :CTi * P]
                                .rearrange("p (c m) -> p c m", m=P),
                            op=OP.is_equal)
                        ps = psA.tile([P, F1], f32, tag="agg")
                        for c in range(CAi):
                            nc.tensor.matmul(ps[:], S[:, c, :],
                                             msgA[:, ca0 + c, :],
                                             start=(c == 0), stop=False)
                        for c in range(CBi):
                            nc.tensor.matmul(ps[:], S[:, CAi + c, :],
                                             msgB[:, cb0 + c, :],
                                             start=False, stop=(c == CBi - 1))
                        ca0 += CAi
                        cb0 += CBi
                        t1 = pool.tile([P, P], f16, tag=tagp + "t1")
                        nc.scalar.mul(t1[:], ps[:], dv_t[:, w:w + 1])
                        psT = psB.tile([P, P], f16, tag="T")
                        nc.tensor.transpose(psT[:], t1[:], idh_t[:])
                        tT = pool.tile([P, P], f16, tag=tagp + "tT")
                        nc.vector.tensor_copy(tT[:], psT[:])
                        ps2 = psB.tile([P, Fout], f32, tag="mm2")
                        nc.tensor.matmul(ps2[:], tT[:], w_t[:],
                                         start=True, stop=True)
                        v = pool.tile([P, Fout], f32, tag=tagp + "v")
                        nc.vector.tensor_tensor(out=v[:], in0=ps2[:],
                                                in1=sh_t[:], op=OP.add)
                        store_cb(w, v)
                    offA += bA
                    offB += bB

            def store_h(w, v):
                u = pool.tile([P, F1], f32, tag="hu")
                nc.scalar.mul(u[:], v[:], dv01_t[:, w:w + 1])
                g = pool.tile([P, F1], f32, tag="hg")
                nc.scalar.mul(g[:], v[:], dv_t[:, w:w + 1])
                hs = pool.tile([P, F1], f16, tag="hs")
                nc.vector.tensor_tensor(out=hs[:], in0=g[:], in1=u[:], op=OP.max)
                nc.sync.dma_start(out=agh[w * P:(w + 1) * P, :], in_=hs[:])

            def store_z(w, v):
                u = pool.tile([P, F2], f32, tag="zu")
                nc.scalar.mul(u[:], v[:], 0.1)
                z = pool.tile([P, Fz], f16, tag="zst")
                nc.vector.memset(z[:, F2:], 0.0)
                nc.vector.tensor_tensor(out=z[:, :F2], in0=v[:], in1=u[:],
                                        op=OP.max)
                nc.sync.dma_start(out=agz[w * P:(w + 1) * P, :], in_=z[:])

            layer(xs_in[0:HALF, :], xs_in[HALF:NP_, :], w1_t, F1, sh1_t,
                  store_h, "a")
            nc.gpsimd.collective_compute(
                "AllGather", mybir.AluOpType.bypass,
                ins=[agh.opt()], outs=[hs_full.opt()], replica_groups=rg)

            layer(hs_full[0:HALF, :], hs_full[HALF:NP_, :], w2_t, F2, sh2_t,
                  store_z, "b")
            nc.gpsimd.collective_compute(
                "AllGather", mybir.AluOpType.bypass,
                ins=[agz.opt()], outs=[z_full.opt()], replica_groups=rg)

            # ---- decode: classes bound statically to z halves ----
            stage_ss = cpool.tile([P, DGT * 16], f32)
            stage_mj = cpool.tile([P, DGT * 16], f32)
            zA = z_full[0:HALF, :]
            zB = z_full[HALF:NP_, :]
            Fp = F2 - 1

            def cls_of(g):
                for k in range(4):
                    if cls_bases[k] <= g < cls_bases[k + 1]:
                        return k
                return 3

            for j in range(DGT // DB):
                g0 = j * DB
                k = cls_of(g0)             # DB groups per batch share a class
                assert cls_of(g0 + DB - 1) == k
                zzS = dpool2.tile([P, DB * 16, Fp], f16, tag="zzS")
                zzD = dpool2.tile([P, DB * 16, F2], f16, tag="zzD")
                si_t = pool.tile([P, DB * 128], i16, tag="dsi")
                di_t = pool.tile([P, DB * 128], i16, tag="ddi")
                nc.sync.dma_start(out=si_t[:],
                                  in_=dsrc_in[:, g0 * 128:(g0 + DB) * 128])
                nc.sync.dma_start(out=di_t[:],
                                  in_=ddst_in[:, g0 * 128:(g0 + DB) * 128])
                _slim_dma_gather(
                    nc.gpsimd, zzS[:, :, :],
                    (zB if k >= 2 else zA)[:, 0:Fp], si_t[:],
                    DB * 2048, Fp, Fz)
                _slim_dma_gather(
                    nc.gpsimd, zzD[:, :, :],
                    (zB if k % 2 else zA)[:, 0:F2], di_t[:],
                    DB * 2048, F2, Fz)
                df = dpool2.tile([P, DB * 16, Fp], f16, tag="ddf")
                nc.vector.tensor_tensor(out=df[:], in0=zzS[:, :, :],
                                        in1=zzD[:, :, 0:Fp], op=OP.subtract)
                sq = dpool2.tile([P, DB * 16, Fp], f16, tag="dsq")
                nc.vector.tensor_tensor(out=sq[:], in0=df[:], in1=df[:],
                                        op=OP.mult)
                nc.vector.reduce_sum(
                    out=stage_ss[:, g0 * 16:(g0 + DB) * 16]
                        .rearrange("p (c o) -> p c o", o=1),
                    in_=sq[:], axis=mybir.AxisListType.X)
                nc.vector.tensor_copy(stage_mj[:, g0 * 16:(g0 + DB) * 16],
                                      zzD[:, :, Fp])
            st_d = cpool.tile([P, DGT * 16], f32)
            nc.scalar.sqrt(st_d[:], stage_ss[:])
            st_v = cpool.tile([P, DGT * 16], f32)
            nc.vector.tensor_tensor(out=st_v[:], in0=stage_mj[:], in1=st_d[:],
                                    op=OP.subtract)
            st_o = cpool.tile([P, DGT * 16], f32)
            nc.scalar.activation(st_o[:], st_v[:], AF.Sigmoid)
            nc.sync.dma_start(out=out_dram[:], in_=st_o[:])
    nc.compile()
    return nc


# --------------------------------------------------------------------------
# public entry
# --------------------------------------------------------------------------
def _prep_inputs(x, edge_index, W1, b1, gamma1, beta1, mean1, var1,
                 W2, b2, gamma2, beta2, mean2, var2, n_cores):
    x = np.asarray(x, np.float32)
    edge_index = np.asarray(edge_index)
    ht = _build_host_tables(x, edge_index, n_cores)
    NP_, NWc, CTmax, DGT = (ht[k] for k in ("NP", "NWc", "CTmax", "DGT"))
    F1 = W1.shape[1]
    F2 = W2.shape[1]

    scale1 = np.asarray(gamma1) / np.sqrt(np.asarray(var1) + EPS)
    shift1 = (np.asarray(beta1) + (np.asarray(b1) - np.asarray(mean1)) * scale1).astype(np.float32)
    W1p = (np.asarray(W1) * scale1[None, :]).astype(np.float16)
    scale2 = np.asarray(gamma2) / np.sqrt(np.asarray(var2) + EPS)
    shift2 = (np.asarray(beta2) + (np.asarray(b2) - np.asarray(mean2)) * scale2).astype(np.float32)
    W2p = (np.asarray(W2) * scale2[None, :]).astype(np.float16)

    xs = np.zeros((NP_, F1), np.float32)
    xs[: ht["N"]] = x
    xs *= ht["dinv"][:, None]
    xs16 = xs.astype(np.float16)

    iota = np.tile(np.arange(P, dtype=np.float16)[None, :], (1, CTmax))
    iota = np.broadcast_to(iota, (P, CTmax * P)).copy()
    identh = np.eye(P, dtype=np.float16)
    sh1_rep = np.broadcast_to(shift1[None, :], (P, F1)).copy()
    sh2_rep = np.broadcast_to(shift2[None, :], (P, F2)).copy()

    in_maps = []
    for c in range(n_cores):
        in_maps.append({
            "xs": xs16, "w1": W1p, "w2": W2p,
            "shift1": sh1_rep, "shift2": sh2_rep,
            "iota": iota, "identh": identh,
            "dv": ht["dv_core"][c],
            "dv01": np.ascontiguousarray(0.1 * ht["dv_core"][c]),
            "idxA": ht["idxA_core"][c], "idxB": ht["idxB_core"][c],
            "dstf": ht["dstf_core"][c],
            "dsrc": ht["dsrc_core"][c], "ddst": ht["ddst_core"][c],
        })
    dims = dict(NP=NP_, NWc=NWc, CAw=ht["CAw"], CBw=ht["CBw"],
                CTmax=ht["CTmax"], doff=ht["doff"], batA=ht["batA"],
                batB=ht["batB"], F1=F1, F2=F2, DGT=DGT,
                batches=ht["batches"], cls_bases=ht["cls_bases"])
    return ht, dims, in_maps


def _assemble_output(ht, results, n_cores):
    E, DGT = ht["E"], ht["DGT"]
    out = np.empty(E, np.float32)
    EC = ht["EC"]
    for c in range(n_cores):
        e0 = c * EC
        arr = results[c]["out"]                         # [P, DGT*16]
        flat = arr.reshape(P, DGT, 16).transpose(1, 2, 0).reshape(-1)
        for base, ek in ht["perm_core"][c]:
            out[e0 + ek] = flat[base:base + len(ek)]
    return out


_program_cache = {}


def _cached_program(dims, n_cores):
    key = (dims["NP"], dims["NWc"], tuple(dims["CAw"]), tuple(dims["CBw"]),
           dims["CTmax"], tuple(dims["doff"]), tuple(dims["batA"]),
           tuple(dims["batB"]), dims["F1"], dims["F2"], dims["DGT"],
           tuple(dims["batches"]), tuple(dims["cls_bases"]), n_cores)
    if key not in _program_cache:
        _program_cache[key] = _build_program(
            dims["NP"], dims["NWc"], dims["CAw"], dims["CBw"],
            dims["CTmax"], dims["doff"], dims["batA"],
            dims["batB"], dims["F1"], dims["F2"], dims["DGT"],
            dims["batches"], dims["cls_bases"], n_cores)
    return _program_cache[key]


def kernel(x, edge_index, W1, b1, gamma1, beta1, mean1, var1,
           W2, b2, gamma2, beta2, mean2, var2, n_cores=8, _trace=False):
    from concourse.bass_utils import run_bass_kernel_spmd

    ht, dims, in_maps = _prep_inputs(
        x, edge_index, W1, b1, gamma1, beta1, mean1, var1,
        W2, b2, gamma2, beta2, mean2, var2, n_cores)
    nc = _cached_program(dims, n_cores)
    try:
        res = run_bass_kernel_spmd(nc, in_maps, list(range(n_cores)), trace=_trace)
    except ModuleNotFoundError:
        res = run_bass_kernel_spmd(nc, in_maps, list(range(n_cores)), trace=False)
    kernel._last_results = res
    kernel._last_nc = nc
    return _assemble_output(ht, res.results, n_cores)


# revision 5
# speedup vs baseline: 1.0727x; 1.0450x over previous
"""GravityAE GNN message-passing kernel for 8 TRN2 NeuronCores (Bass/Tile), v3.

Math (GCN autoencoder, eval):
  scale_k = gamma_k/sqrt(var_k+eps); shift_k = beta_k + (b_k-mean_k)*scale_k
  Wkp = W_k*scale_k;  dinv[n] = 1/sqrt(in_deg incl self loop)
  xs = dinv*x  (fp16 DRAM table);  agg1[d] = sum_{e:dst=d} xs[src]
  h  = leaky(dinv_d*agg1 @ W1p + shift1);  hs = dinv*h  (fp16 table)
  z  = leaky(dinv_d*(sum hs[src]) @ W2p + shift2)
  out[e] = sigmoid(z[dst,64] - ||z[src,:64]-z[dst,:64]||)

Distribution: dst-sharded aggregation (49 windows of 128 nodes per core;
edges+self-loops sorted by dst, then split per window into two compacted
streams by src half since dma_gather indices are int16). Gathers use the
Pool-engine dma_gather ucode (one call per 4-window batch per half,
single_packet=False) which amortizes the ~1us SWDGE fixed cost over
thousands of descriptors. Segment-sum = S^T @ msg in PSUM with S built
by one fp16 is_equal per window; dense W matmul on the PE-transposed
window; leaky-relu as max(dinv*v, 0.1*dinv*v) with the scales applied by
the Act engine. AllGather (x8) rebuilds the hs and z tables between
stages; z is stored padded to 128 cols so decode rows are 256B. Decode
is edge-sharded, edges classed by (src-half, dst-half) so each 2-group
batch needs two dma_gathers; the host inverse-permutes the output.
"""
import numpy as np

P = 128
EPS = 1e-5
HALF = 32768          # int16 index limit for dma_gather
GK = 2                # windows per layer gather batch
DB = 2                # decode groups per gather batch


# --------------------------------------------------------------------------
# host-side preprocessing
# --------------------------------------------------------------------------
def _idx16(idx_flat, cols):
    """int16 index tile [128, cols]: slot i -> [i%16, i//16], replicated x8."""
    t = np.zeros((16, cols), np.int16)
    n = len(idx_flat)
    t[np.arange(n) % 16, np.arange(n) // 16] = idx_flat.astype(np.int16)
    return np.tile(t, (8, 1))


def _build_host_tables(x, edge_index, n_cores):
    N = x.shape[0]
    E = edge_index.shape[1]
    NW = ((N + P - 1) // P + n_cores - 1) // n_cores * n_cores
    NP_ = NW * P
    src = edge_index[0].astype(np.int64)
    dst = edge_index[1].astype(np.int64)
    s_all = np.concatenate([src, np.arange(N)])
    d_all = np.concatenate([dst, np.arange(N)])
    deg = np.bincount(d_all, minlength=NP_).astype(np.float64)
    dinv = np.zeros(NP_, np.float32)
    nz = deg > 0
    dinv[nz] = (1.0 / np.sqrt(deg[nz])).astype(np.float32)

    # sort by (dst window, src-half): gives per-window contiguous A then B runs
    win = d_all // P
    half = (s_all >= HALF).astype(np.int64)
    order = np.lexsort((half, d_all))
    s_sorted = s_all[order]
    d_sorted = d_all[order]
    h_sorted = half[order]

    cntAB = np.bincount(win[order] * 2 + h_sorted, minlength=NW * 2).reshape(NW, 2)
    NWc = NW // n_cores
    cpc = cntAB.reshape(n_cores, NWc, 2)
    CAw = np.maximum(1, np.ceil(cpc[:, :, 0].max(0) / P).astype(np.int64))  # [NWc]
    CBw = np.maximum(1, np.ceil(cpc[:, :, 1].max(0) / P).astype(np.int64))
    CTw = CAw + CBw
    CTmax = int(CTw.max())
    doff = np.zeros(NWc + 1, np.int64)
    np.cumsum(CTw, out=doff[1:])                      # dstf col offsets

    # per-window slot streams (per core, window-slot i): A then B
    starts = np.zeros(NW + 1, np.int64)
    np.cumsum(cntAB.sum(1), out=starts[1:])

    nbat = (NWc + GK - 1) // GK
    batches = [min(GK, NWc - b * GK) for b in range(nbat)]
    batA = [int(CAw[b * GK:b * GK + k].sum()) for b, k in enumerate(batches)]
    batB = [int(CBw[b * GK:b * GK + k].sum()) for b, k in enumerate(batches)]

    idxA_core, idxB_core, dstf_core, dv_core = [], [], [], []
    dv = dinv.reshape(NW, P).T                        # [P, NW]
    for c in range(n_cores):
        segsA, segsB = [], []
        dstf_c = np.full((int(doff[-1]), P), -1.0, np.float16)  # [cols, P]
        for b, k in enumerate(batches):
            ia = np.zeros(batA[b] * P, np.int64)
            ib = np.zeros(batB[b] * P, np.int64)
            oa = ob = 0
            for i in range(b * GK, b * GK + k):
                W = c * NWc + i
                sl = slice(starts[W], starts[W + 1])
                ss = s_sorted[sl]
                dd = d_sorted[sl] - W * P
                hh = h_sorted[sl]
                a, bb = ss[hh == 0], ss[hh == 1] - HALF
                ia[oa:oa + len(a)] = a
                ib[ob:ob + len(bb)] = bb
                da, db_ = dd[hh == 0], dd[hh == 1]
                col0 = doff[i]
                dstf_c.reshape(-1)[col0 * P:col0 * P + len(da)] = da.astype(np.float16)
                colB = (doff[i] + CAw[i]) * P
                dstf_c.reshape(-1)[colB:colB + len(db_)] = db_.astype(np.float16)
                oa += CAw[i] * P
                ob += CBw[i] * P
            segsA.append(_idx16(ia, batA[b] * 8))
            segsB.append(_idx16(ib, batB[b] * 8))
        idxA_core.append(np.concatenate(segsA, 1))
        idxB_core.append(np.concatenate(segsB, 1))
        dstf_core.append(np.ascontiguousarray(dstf_c.T))   # [P, sum(CTw)]
        dv_core.append(np.ascontiguousarray(dv[:, c * NWc:(c + 1) * NWc]))

    # ---- decode: 4 classes by (src-half, dst-half), groups of 2048 ----
    EC = (E + n_cores - 1) // n_cores
    cls_all = (src >= HALF) * 2 + (dst >= HALF)
    gcls = np.zeros((n_cores, 4), np.int64)
    per_core = []
    for c in range(n_cores):
        e0, e1 = c * EC, min((c + 1) * EC, E)
        ids = [np.nonzero(cls_all[e0:e1] == k)[0] for k in range(4)]
        gcls[c] = [(len(i) + 2047) // 2048 for i in ids]
        per_core.append((e0, ids))
    gmax = gcls.max(0)                                # uniform group counts
    gmax = np.maximum(gmax, 1)
    # pad each class to a DB multiple so every DB-batch is single-class
    gmax = (gmax + DB - 1) // DB * DB
    DGT = int(gmax.sum())
    g0 = np.zeros(5, np.int64)
    np.cumsum(gmax, out=g0[1:])

    dsrc_core, ddst_core, perm_core = [], [], []
    for c in range(n_cores):
        e0, ids = per_core[c]
        sflat = np.zeros(DGT * 2048, np.int64)
        dflat = np.zeros(DGT * 2048, np.int64)
        perm = []                                     # (slot, local_edge_id)
        for k in range(4):
            base = g0[k] * 2048
            ek = ids[k]
            sk = src[e0 + ek]
            dk = dst[e0 + ek]
            sflat[base:base + len(ek)] = sk - (HALF if k >= 2 else 0)
            dflat[base:base + len(ek)] = dk - (HALF if k % 2 else 0)
            perm.append((base, ek))
        dsrc_core.append(_idx16(sflat, DGT * 128))
        ddst_core.append(_idx16(dflat, DGT * 128))
        perm_core.append(perm)

    return dict(N=N, E=E, NW=NW, NP=NP_, CAw=CAw, CBw=CBw, CTmax=CTmax,
                doff=doff, batA=batA, batB=batB, NWc=NWc,
                EC=EC, DGT=DGT, batches=batches, cls_bases=g0,
                dinv=dinv, idxA_core=idxA_core, idxB_core=idxB_core,
                dstf_core=dstf_core, dv_core=dv_core,
                dsrc_core=dsrc_core, ddst_core=ddst_core,
                perm_core=perm_core)


def _slim_dma_gather(g, out_ap, in_ap, idxs_ap, num_idxs, elem_size, elem_step):
    """dma_gather with elem_size < row stride (stride must be a 256B multiple).

    Mirrors BassGpsimd.dma_gather's lowering but permits sub-256B elements,
    which the non-transpose ucode path handles (HW-verified)."""
    import concourse.mybir as mybir
    import concourse.ap_utils as ap_utils
    from concourse._compat import exact_div
    assert idxs_ap.dtype == mybir.dt.int16
    assert in_ap.dtype == out_ap.dtype
    assert ap_utils.ap_is_contiguous(out_ap.ap[1:])
    assert ap_utils.ap_is_contiguous(idxs_ap.ap[1:])
    assert in_ap.ap[0][0] == elem_step
    stride_bytes_256 = exact_div(elem_step * mybir.dt.size(in_ap.dtype), 256)
    _in_ap = g.lower_ap_dma(in_ap, for_custom_bir_dma=True)
    _idxs_ap = g.lower_ap(idxs_ap)
    _out_ap = g.lower_ap(out_ap)
    return g.add_instruction(
        mybir.InstDMAGatherAnt(
            name=g.bass.get_next_instruction_name(),
            ins=[*_in_ap, _idxs_ap, g.lower_val_access(g.to_reg(num_idxs))],
            outs=[_out_ap],
            transpose=False, num_idxs=num_idxs, elem_size=elem_size,
            stride_bytes_256=stride_bytes_256, gen_mode=0,
            single_packet=False, queue_num=0, sbuf_tokens_per_rank=0,
            sbuf_free_dim_per_rank=0, sbuf_free_dim_pad_per_rank=0,
            sbuf_byte_offset=0))


# --------------------------------------------------------------------------
# bass program
# --------------------------------------------------------------------------
def _build_program(NP_, NWc, CAw, CBw, CTmax, doff, batA, batB, F1, F2, DGT,
                   batches, cls_bases, n_cores):
    import concourse.bass as bass
    import concourse.tile as tile
    from concourse import bacc, mybir

    dt = mybir.dt
    f32 = dt.float32
    f16 = dt.float16
    i16 = dt.int16
    Nc = NWc * P
    SA, SB, ST = sum(batA), sum(batB), int(doff[-1])
    mA_max, mB_max = max(batA), max(batB)
    Fz = 128                      # z stored padded to 128 cols (256B rows)
    NB = NP_ - HALF               # rows in half B

    nc = bacc.Bacc("TRN2", target_bir_lowering=False, debug=False,
                   num_devices=n_cores)
    xs_in = nc.declare_dram_parameter("xs", [NP_, F1], f16, isOutput=False)
    w1_in = nc.declare_dram_parameter("w1", [F1, F1], f16, isOutput=False)
    w2_in = nc.declare_dram_parameter("w2", [F1, F2], f16, isOutput=False)
    sh1_in = nc.declare_dram_parameter("shift1", [P, F1], f32, isOutput=False)
    sh2_in = nc.declare_dram_parameter("shift2", [P, F2], f32, isOutput=False)
    iota_in = nc.declare_dram_parameter("iota", [P, CTmax * P], f16, isOutput=False)
    idh_in = nc.declare_dram_parameter("identh", [P, P], f16, isOutput=False)
    dv_in = nc.declare_dram_parameter("dv", [P, NWc], f32, isOutput=False)
    dv01_in = nc.declare_dram_parameter("dv01", [P, NWc], f32, isOutput=False)
    ixa_in = nc.declare_dram_parameter("idxA", [P, SA * 8], i16, isOutput=False)
    ixb_in = nc.declare_dram_parameter("idxB", [P, SB * 8], i16, isOutput=False)
    dstf_in = nc.declare_dram_parameter("dstf", [P, ST], f16, isOutput=False)
    dsrc_in = nc.declare_dram_parameter("dsrc", [P, DGT * 128], i16, isOutput=False)
    ddst_in = nc.declare_dram_parameter("ddst", [P, DGT * 128], i16, isOutput=False)
    out_dram = nc.declare_dram_parameter("out", [P, DGT * 16], f32, isOutput=True)

    rg = [list(range(n_cores))]
    AF = mybir.ActivationFunctionType
    OP = mybir.AluOpType

    with tile.TileContext(nc) as tc:
        with (
            tc.tile_pool(name="const", bufs=1) as cpool,
            tc.tile_pool(name="msgp", bufs=3) as mpool,
            tc.tile_pool(name="sbuf", bufs=4) as pool,
            tc.tile_pool(name="dec", bufs=2) as dpool2,
            tc.tile_pool(name="psA", bufs=4, space="PSUM") as psA,
            tc.tile_pool(name="psB", bufs=2, space="PSUM") as psB,
            tc.tile_pool(name="dram", bufs=1, space="DRAM") as dpool,
        ):
            # ---- constants ----
            w1_t = cpool.tile([F1, F1], f16)
            w2_t = cpool.tile([F1, F2], f16)
            sh1_t = cpool.tile([P, F1], f32)
            sh2_t = cpool.tile([P, F2], f32)
            iota_t = cpool.tile([P, CTmax * P], f16)
            idh_t = cpool.tile([P, P], f16)
            dv_t = cpool.tile([P, NWc], f32)
            dv01_t = cpool.tile([P, NWc], f32)
            ixa_t = cpool.tile([P, SA * 8], i16)
            ixb_t = cpool.tile([P, SB * 8], i16)
            dstf_t = cpool.tile([P, ST], f16)
            for t_, p_ in ((w1_t, w1_in), (w2_t, w2_in), (sh1_t, sh1_in),
                           (sh2_t, sh2_in), (iota_t, iota_in), (idh_t, idh_in),
                           (dv_t, dv_in), (dv01_t, dv01_in), (ixa_t, ixa_in),
                           (ixb_t, ixb_in), (dstf_t, dstf_in)):
                nc.sync.dma_start(out=t_[:], in_=p_[:])

            # ---- collective buffers ----
            agh = dpool.tile([Nc, F1], f16)
            hs_full = dpool.tile([NP_, F1], f16, addr_space="Shared")
            agz = dpool.tile([Nc, Fz], f16)
            z_full = dpool.tile([NP_, Fz], f16, addr_space="Shared")

            # ---- one GCN layer ----
            def layer(tabA, tabB, w_t, Fout, sh_t, store_cb, tagp):
                offA = offB = 0
                for b, K in enumerate(batches):
                    w0 = b * GK
                    bA, bB = batA[b], batB[b]
                    msgA = mpool.tile([P, mA_max, F1], f16, tag=tagp + "mA")
                    msgB = mpool.tile([P, mB_max, F1], f16, tag=tagp + "mB")
                    nc.gpsimd.dma_gather(
                        out_ap=msgA[:, :bA, :], in_ap=tabA,
                        idxs_ap=ixa_t[:, offA * 8:(offA + bA) * 8],
                        num_idxs=bA * P, num_idxs_reg=bA * P,
                        elem_size=F1, single_packet=False)
                    nc.gpsimd.dma_gather(
                        out_ap=msgB[:, :bB, :], in_ap=tabB,
                        idxs_ap=ixb_t[:, offB * 8:(offB + bB) * 8],
                        num_idxs=bB * P, num_idxs_reg=bB * P,
                        elem_size=F1, single_packet=False)
                    ca0 = cb0 = 0
                    for kw in range(K):
                        w = w0 + kw
                        CAi, CBi = int(CAw[w]), int(CBw[w])
                        CTi = CAi + CBi
                        d0 = int(doff[w])
                        S = pool.tile([P, CTmax, P], f16, tag=tagp + "S")
                        nc.vector.tensor_tensor(
                            out=S[:, :CTi, :],
                            in0=dstf_t[:, d0:d0 + CTi]
                                .rearrange("p (c o) -> p c o", o=1)
                                .to_broadcast([P, CTi, P]),
                            in1=iota_t[:, :CTi * P]
                                .rearrange("p (c m) -> p c m", m=P),
                            op=OP.is_equal)
                        ps = psA.tile([P, F1], f32, tag="agg")
                        for c in range(CAi):
                            nc.tensor.matmul(ps[:], S[:, c, :],
                                             msgA[:, ca0 + c, :],
                                             start=(c == 0), stop=False)
                        for c in range(CBi):
                            nc.tensor.matmul(ps[:], S[:, CAi + c, :],
                                             msgB[:, cb0 + c, :],
                                             start=False, stop=(c == CBi - 1))
                        ca0 += CAi
                        cb0 += CBi
                        t1 = pool.tile([P, P], f16, tag=tagp + "t1")
                        nc.scalar.mul(t1[:], ps[:], dv_t[:, w:w + 1])
                        psT = psB.tile([P, P], f16, tag="T")
                        nc.tensor.transpose(psT[:], t1[:], idh_t[:])
                        tT = pool.tile([P, P], f16, tag=tagp + "tT")
                        nc.vector.tensor_copy(tT[:], psT[:])
                        ps2 = psB.tile([P, Fout], f32, tag="mm2")
                        nc.tensor.matmul(ps2[:], tT[:], w_t[:],
                                         start=True, stop=True)
                        v = pool.tile([P, Fout], f32, tag=tagp + "v")
                        nc.vector.tensor_tensor(out=v[:], in0=ps2[:],
                                                in1=sh_t[:], op=OP.add)
                        store_cb(w, v)
                    offA += bA
                    offB += bB

            def store_h(w, v):
                u = pool.tile([P, F1], f32, tag="hu")
                nc.scalar.mul(u[:], v[:], dv01_t[:, w:w + 1])
                g = pool.tile([P, F1], f32, tag="hg")
                nc.scalar.mul(g[:], v[:], dv_t[:, w:w + 1])
                hs = pool.tile([P, F1], f16, tag="hs")
                nc.vector.tensor_tensor(out=hs[:], in0=g[:], in1=u[:], op=OP.max)
                nc.sync.dma_start(out=agh[w * P:(w + 1) * P, :], in_=hs[:])

            def store_z(w, v):
                u = pool.tile([P, F2], f32, tag="zu")
                nc.scalar.mul(u[:], v[:], 0.1)
                z = pool.tile([P, Fz], f16, tag="zst")
                nc.vector.memset(z[:, F2:], 0.0)
                nc.vector.tensor_tensor(out=z[:, :F2], in0=v[:], in1=u[:],
                                        op=OP.max)
                nc.sync.dma_start(out=agz[w * P:(w + 1) * P, :], in_=z[:])

            layer(xs_in[0:HALF, :], xs_in[HALF:NP_, :], w1_t, F1, sh1_t,
                  store_h, "a")
            nc.gpsimd.collective_compute(
                "AllGather", mybir.AluOpType.bypass,
                ins=[agh.opt()], outs=[hs_full.opt()], replica_groups=rg)

            layer(hs_full[0:HALF, :], hs_full[HALF:NP_, :], w2_t, F2, sh2_t,
                  store_z, "b")
            nc.gpsimd.collective_compute(
                "AllGather", mybir.AluOpType.bypass,
                ins=[agz.opt()], outs=[z_full.opt()], replica_groups=rg)

            # ---- decode: classes bound statically to z halves ----
            stage_ss = cpool.tile([P, DGT * 16], f32)
            stage_mj = cpool.tile([P, DGT * 16], f32)
            zA = z_full[0:HALF, :]
            zB = z_full[HALF:NP_, :]
            Fp = F2 - 1

            def cls_of(g):
                for k in range(4):
                    if cls_bases[k] <= g < cls_bases[k + 1]:
                        return k
                return 3

            for j in range(DGT // DB):
                g0 = j * DB
                k = cls_of(g0)             # DB groups per batch share a class
                assert cls_of(g0 + DB - 1) == k
                zzS = dpool2.tile([P, DB * 16, Fp], f16, tag="zzS")
                zzD = dpool2.tile([P, DB * 16, F2], f16, tag="zzD")
                si_t = pool.tile([P, DB * 128], i16, tag="dsi")
                di_t = pool.tile([P, DB * 128], i16, tag="ddi")
                nc.sync.dma_start(out=si_t[:],
                                  in_=dsrc_in[:, g0 * 128:(g0 + DB) * 128])
                nc.sync.dma_start(out=di_t[:],
                                  in_=ddst_in[:, g0 * 128:(g0 + DB) * 128])
                _slim_dma_gather(
                    nc.gpsimd, zzS[:, :, :],
                    (zB if k >= 2 else zA)[:, 0:Fp], si_t[:],
                    DB * 2048, Fp, Fz)
                _slim_dma_gather(
                    nc.gpsimd, zzD[:, :, :],
                    (zB if k % 2 else zA)[:, 0:F2], di_t[:],
                    DB * 2048, F2, Fz)
                df = dpool2.tile([P, DB * 16, Fp], f16, tag="ddf")
                nc.vector.tensor_tensor(out=df[:], in0=zzS[:, :, :],
                                        in1=zzD[:, :, 0:Fp], op=OP.subtract)
                sq = dpool2.tile([P, DB * 16, Fp], f16, tag="dsq")
                nc.vector.tensor_tensor(out=sq[:], in0=df[:], in1=df[:],
                                        op=OP.mult)
                nc.vector.reduce_sum(
                    out=stage_ss[:, g0 * 16:(g0 + DB) * 16]
                        .rearrange("p (c o) -> p c o", o=1),
                    in_=sq[:], axis=mybir.AxisListType.X)
                nc.vector.tensor_copy(stage_mj[:, g0 * 16:(g0 + DB) * 16],
                                      zzD[:, :, Fp])
            st_d = cpool.tile([P, DGT * 16], f32)
            nc.scalar.sqrt(st_d[:], stage_ss[:])
            st_v = cpool.tile([P, DGT * 16], f32)
            nc.vector.tensor_tensor(out=st_v[:], in0=stage_mj[:], in1=st_d[:],
                                    op=OP.subtract)
            st_o = cpool.tile([P, DGT * 16], f32)
            nc.scalar.activation(st_o[:], st_v[:], AF.Sigmoid)
            nc.sync.dma_start(out=out_dram[:], in_=st_o[:])
    nc.compile()
    return nc


# --------------------------------------------------------------------------
# public entry
# --------------------------------------------------------------------------
def _prep_inputs(x, edge_index, W1, b1, gamma1, beta1, mean1, var1,
                 W2, b2, gamma2, beta2, mean2, var2, n_cores):
    x = np.asarray(x, np.float32)
    edge_index = np.asarray(edge_index)
    ht = _build_host_tables(x, edge_index, n_cores)
    NP_, NWc, CTmax, DGT = (ht[k] for k in ("NP", "NWc", "CTmax", "DGT"))
    F1 = W1.shape[1]
    F2 = W2.shape[1]

    scale1 = np.asarray(gamma1) / np.sqrt(np.asarray(var1) + EPS)
    shift1 = (np.asarray(beta1) + (np.asarray(b1) - np.asarray(mean1)) * scale1).astype(np.float32)
    W1p = (np.asarray(W1) * scale1[None, :]).astype(np.float16)
    scale2 = np.asarray(gamma2) / np.sqrt(np.asarray(var2) + EPS)
    shift2 = (np.asarray(beta2) + (np.asarray(b2) - np.asarray(mean2)) * scale2).astype(np.float32)
    W2p = (np.asarray(W2) * scale2[None, :]).astype(np.float16)

    xs = np.zeros((NP_, F1), np.float32)
    xs[: ht["N"]] = x
    xs *= ht["dinv"][:, None]
    xs16 = xs.astype(np.float16)

    iota = np.tile(np.arange(P, dtype=np.float16)[None, :], (1, CTmax))
    iota = np.broadcast_to(iota, (P, CTmax * P)).copy()
    identh = np.eye(P, dtype=np.float16)
    sh1_rep = np.broadcast_to(shift1[None, :], (P, F1)).copy()
    sh2_rep = np.broadcast_to(shift2[None, :], (P, F2)).copy()

    in_maps = []
    for c in range(n_cores):
        in_maps.append({
            "xs": xs16, "w1": W1p, "w2": W2p,
            "shift1": sh1_rep, "shift2": sh2_rep,
            "iota": iota, "identh": identh,
            "dv": ht["dv_core"][c],
            "dv01": np.ascontiguousarray(0.1 * ht["dv_core"][c]),
            "idxA": ht["idxA_core"][c], "idxB": ht["idxB_core"][c],
            "dstf": ht["dstf_core"][c],
            "dsrc": ht["dsrc_core"][c], "ddst": ht["ddst_core"][c],
        })
    dims = dict(NP=NP_, NWc=NWc, CAw=ht["CAw"], CBw=ht["CBw"],
                CTmax=ht["CTmax"], doff=ht["doff"], batA=ht["batA"],
                batB=ht["batB"], F1=F1, F2=F2, DGT=DGT,
                batches=ht["batches"], cls_bases=ht["cls_bases"])
    return ht, dims, in_maps


def _assemble_output(ht, results, n_cores):
    E, DGT = ht["E"], ht["DGT"]
    out = np.empty(E, np.float32)
    EC = ht["EC"]
    for c in range(n_cores):
        e0 = c * EC
        arr = results[c]["out"]                         # [P, DGT*16]
        flat = arr.reshape(P, DGT, 16).transpose(1, 2, 0).reshape(-1)
        for base, ek in ht["perm_core"][c]:
            out[e0 + ek] = flat[base:base + len(ek)]
    return out


_program_cache = {}


def _cached_program(dims, n_cores):
    key = (dims["NP"], dims["NWc"], tuple(dims["CAw"]), tuple(dims["CBw"]),
           dims["CTmax"], tuple(dims["doff"]), tuple(dims["batA"]),
           tuple(dims["batB"]), dims["F1"], dims["F2"], dims["DGT"],
           tuple(dims["batches"]), tuple(dims["cls_bases"]), n_cores)
    if key not in _program_cache:
        _program_cache[key] = _build_program(
            dims["NP"], dims["NWc"], dims["CAw"], dims["CBw"],
            dims["CTmax"], dims["doff"], dims["batA"],
            dims["batB"], dims["F1"], dims["F2"], dims["DGT"],
            dims["batches"], dims["cls_bases"], n_cores)
    return _program_cache[key]


def kernel(x, edge_index, W1, b1, gamma1, beta1, mean1, var1,
           W2, b2, gamma2, beta2, mean2, var2, n_cores=8, _trace=False):
    from concourse.bass_utils import run_bass_kernel_spmd

    ht, dims, in_maps = _prep_inputs(
        x, edge_index, W1, b1, gamma1, beta1, mean1, var1,
        W2, b2, gamma2, beta2, mean2, var2, n_cores)
    nc = _cached_program(dims, n_cores)
    try:
        res = run_bass_kernel_spmd(nc, in_maps, list(range(n_cores)), trace=_trace)
    except ModuleNotFoundError:
        res = run_bass_kernel_spmd(nc, in_maps, list(range(n_cores)), trace=False)
    kernel._last_results = res
    kernel._last_nc = nc
    return _assemble_output(ht, res.results, n_cores)
